# revision 1
# baseline (speedup 1.0000x reference)
"""CrossTransformer Trainium2 kernel.

Shapes (hardcoded): B=4, TQ=TK=1024, D=1024, H=16, DK=DV=64.
Sharding: 8 cores = 4 batches x 2 query-row halves. Each core computes
out[b, qs:qs+512, :] independently (k/v work duplicated across the pair
of cores sharing a batch; no collectives).

Weights are converted to bf16 on the host (the matmuls are bf16 either
way; converting host-side halves the weight DMA traffic and removes 48
on-device cast ops).
"""
import sys

for _p in ("/root/.axon_site", "/root/.axon_site/_ro/trn_rl_repo",
           "/root/.axon_site/_ro/pypackages", "/opt/trn_rl_repo"):
    if _p not in sys.path:
        sys.path.append(_p)

import numpy as np
import ml_dtypes
import concourse.bass as bass
from concourse import bacc
import concourse.tile as tile
import concourse.mybir as mybir
from concourse.masks import make_identity
from concourse.bass_utils import run_bass_kernel_spmd

F32 = mybir.dt.float32
BF = mybir.dt.bfloat16
AF = mybir.ActivationFunctionType
OP = mybir.AluOpType

B, TQ, TK, D = 4, 1024, 1024, 1024
H, DK, DV = 16, 64, 64
TQC = TQ // 2          # 512 query rows per core
NT = TQC // 128        # 4 q-row tiles
KD = D // 128          # 8 contraction chunks
MASK_NEG = -30000.0

WEIGHT_NAMES = ["q_w", "k_w", "v_w", "o_w", "l1_w", "l2_w"]
VEC_NAMES = ["q_b", "k_b", "v_b", "o_b", "l1_b", "l2_b",
             "ln1_g", "ln1_b", "ln2_g", "ln2_b",
             "mln1_g", "mln1_b", "mln2_g", "mln2_b"]


def build_kernel(compile=True, repeat=1, stop_after=None):
    nc = bacc.Bacc()
    xs = nc.dram_tensor("xs", (TQC, D), F32, kind="ExternalInput")
    y = nc.dram_tensor("y", (TK, D), F32, kind="ExternalInput")
    mb = nc.dram_tensor("mb", (TK,), F32, kind="ExternalInput")
    wd = {n: nc.dram_tensor(n, (D, D), BF, kind="ExternalInput") for n in WEIGHT_NAMES}
    vd = {n: nc.dram_tensor(n, (D,), F32, kind="ExternalInput") for n in VEC_NAMES}
    out = nc.dram_tensor("out", (TQC, D), F32, kind="ExternalOutput")

    with tile.TileContext(nc) as tc:
        for r in range(repeat):
            _emit(nc, tc, xs, y, mb, wd, vd, out, pfx=f"r{r}_", stop_after=stop_after)
    if compile:
        nc.compile()
    return nc


def _emit(nc, tc, xs, y, mb, wd, vd, out, pfx="", stop_after=None):
    from contextlib import ExitStack

    ctx = ExitStack()
    with ctx:
        persist = ctx.enter_context(tc.tile_pool(name=pfx + "persist", bufs=1))
        lnp = ctx.enter_context(tc.tile_pool(name=pfx + "lnp", bufs=2))
        bcast = ctx.enter_context(tc.tile_pool(name=pfx + "bcast", bufs=4))
        wts = ctx.enter_context(tc.tile_pool(name=pfx + "wts", bufs=2))
        psmm = ctx.enter_context(tc.tile_pool(name=pfx + "psmm", bufs=4, space="PSUM"))
        psmi = ctx.enter_context(tc.tile_pool(name=pfx + "psmi", bufs=2, space="PSUM"))

        # ---------------- setup constants ----------------
        ident = persist.tile([128, 128], BF, tag="ident", name=pfx + "ident")
        make_identity(nc, ident)
        eps_t = persist.tile([128, 1], F32, tag="eps", name=pfx + "eps")
        nc.vector.memset(eps_t[:], 1e-5)
        ones_c = persist.tile([128, DV], BF, tag="ones_c", name=pfx + "ones_c")
        nc.vector.memset(ones_c[:], 1.0)
        mb_sb = persist.tile([128, KD], F32, tag="mb_sb", name=pfx + "mb_sb")
        nc.sync.dma_start(mb_sb[:], mb.rearrange("(t p) -> p t", p=128))
        bq_sb = persist.tile([128, KD], F32, tag="bq_sb", name=pfx + "bq_sb")
        nc.sync.dma_start(bq_sb[:], vd["q_b"].rearrange("(t p) -> p t", p=128))
        bk_sb = persist.tile([128, KD], F32, tag="bk_sb", name=pfx + "bk_sb")
        nc.sync.dma_start(bk_sb[:], vd["k_b"].rearrange("(t p) -> p t", p=128))

        def bcast_tile(name):
            t = bcast.tile([128, D], F32, tag="bc", name=pfx + f"bc_{name}")
            nc.sync.dma_start(t[:], vd[name][:].unsqueeze(0).partition_broadcast(128))
            return t

        def load_weight(name):
            wt = wts.tile([128, KD, D], BF, tag="wbf", name=pfx + f"wbf_{name}")
            nc.sync.dma_start(wt[:], wd[name].rearrange("(ko p) n -> p ko n", p=128))
            return wt

        # LN(+affine)+ReLU: src [128, D] f32 -> dst [128, D] bf16
        def ln_relu(src, gt, bt, dst, key):
            stats = lnp.tile([128, 2, 6], F32, tag="stats", name=pfx + f"st_{key}")
            for i in range(2):
                nc.vector.bn_stats(stats[:, i, :], src[:, i * 512:(i + 1) * 512])
            mv = lnp.tile([128, 2], F32, tag="mv", name=pfx + f"mv_{key}")
            nc.vector.bn_aggr(mv[:], stats[:])
            std = lnp.tile([128, 1], F32, tag="std", name=pfx + f"sd_{key}")
            nc.scalar.activation(std[:], mv[:, 1:2], AF.Sqrt, bias=eps_t[:], scale=1.0)
            rstd = lnp.tile([128, 1], F32, tag="rstd", name=pfx + f"rs_{key}")
            nc.vector.reciprocal(rstd[:], std[:])
            z = lnp.tile([128, D], F32, tag="lnz", name=pfx + f"z_{key}")
            nc.vector.tensor_scalar(z[:], src[:], mv[:, 0:1], rstd[:],
                                    OP.subtract, OP.mult)
            nc.gpsimd.tensor_tensor(z[:], z[:], gt[:], OP.mult)
            nc.gpsimd.tensor_tensor(z[:], z[:], bt[:], OP.add)
            nc.scalar.activation(dst, z[:], AF.Relu, scale=1.0)

        # PE transpose of a [128,128] bf16 block; psum->sbuf copy on DVE
        tr_count = [0]

        def transpose_128(src_ap, dst_ap):
            pt = psmi.tile([128, 128], BF, tag="ps_tr", name=pfx + f"tr{tr_count[0]}")
            nc.tensor.transpose(pt[:], src_ap, ident[:])
            nc.vector.tensor_copy(dst_ap, pt[:])
            tr_count[0] += 1

        x_sb = persist.tile([128, NT, D], F32, tag="x_sb", name=pfx + "x_sb")
        qT = persist.tile([128, KD, TQC], BF, tag="qT", name=pfx + "qT")
        kT = persist.tile([128, KD, TK], BF, tag="kT", name=pfx + "kT")
        v_ext = persist.tile([128, KD, H, DV + 1], BF, tag="v_ext", name=pfx + "v_ext")
        attnT = persist.tile([128, KD, TQC], BF, tag="attnT", name=pfx + "attnT")

        with (
            tc.tile_pool(name=pfx + "pab", bufs=1) as pab,
            tc.tile_pool(name=pfx + "ldp", bufs=2) as ldp,
            tc.tile_pool(name=pfx + "ptr", bufs=1) as ptr,
        ):
            # ---------------- phase A: LN + relu ----------------
            g1 = bcast_tile("ln1_g")
            b1 = bcast_tile("ln1_b")
            g2 = bcast_tile("ln2_g")
            b2 = bcast_tile("ln2_b")

            x1 = pab.tile([128, NT, D], BF, tag="x1", name=pfx + "x1")
            for t in range(NT):
                nc.sync.dma_start(x_sb[:, t, :],
                                  xs.rearrange("(t p) d -> p t d", p=128)[:, t, :])
                ln_relu(x_sb[:, t, :], g1, b1, x1[:, t, :], f"x{t}")

            y1 = pab.tile([128, KD, D], BF, tag="y1", name=pfx + "y1")
            for t in range(KD):
                yl = ldp.tile([128, D], F32, tag="yload", name=pfx + f"yl_{t}")
                nc.sync.dma_start(yl[:], y.rearrange("(t p) d -> p t d", p=128)[:, t, :])
                ln_relu(yl[:], g2, b2, y1[:, t, :], f"y{t}")

            if stop_after == "A":
                return
            # ---------------- phase B: transposes ----------------
            x1T = ptr.tile([128, KD, TQC], BF, tag="x1T", name=pfx + "x1T")
            for dt in range(KD):
                for tt in range(NT):
                    transpose_128(x1[:, tt, dt * 128:(dt + 1) * 128],
                                  x1T[:, dt, tt * 128:(tt + 1) * 128])
            y1T = ptr.tile([128, KD, TK], BF, tag="y1T", name=pfx + "y1T")
            for dt in range(KD):
                for tt in range(KD):
                    transpose_128(y1[:, tt, dt * 128:(dt + 1) * 128],
                                  y1T[:, dt, tt * 128:(tt + 1) * 128])

            if stop_after == "B":
                return
            # ---------------- phase C: projections ----------------
            # qT[hdk, tq] = q_w.T @ x1T
            wq = load_weight("q_w")
            for m in range(KD):
                pq = psmm.tile([128, TQC], F32, tag="ps_mm", name=pfx + f"pq{m}")
                for kc in range(KD):
                    nc.tensor.matmul(pq[:], wq[:, kc, m * 128:(m + 1) * 128],
                                     x1T[:, kc, :],
                                     start=(kc == 0), stop=(kc == KD - 1))
                nc.scalar.activation(qT[:, m, :], pq[:], AF.Identity,
                                     bias=bq_sb[:, m:m + 1], scale=1.0)

            # kT[hdk, tk] = k_w.T @ y1T
            wk = load_weight("k_w")
            for m in range(KD):
                for nt2 in range(2):
                    pk = psmm.tile([128, 512], F32, tag="ps_mm", name=pfx + f"pk{m}_{nt2}")
                    for kc in range(KD):
                        nc.tensor.matmul(pk[:], wk[:, kc, m * 128:(m + 1) * 128],
                                         y1T[:, kc, nt2 * 512:(nt2 + 1) * 512],
                                         start=(kc == 0), stop=(kc == KD - 1))
                    nc.scalar.activation(kT[:, m, nt2 * 512:(nt2 + 1) * 512], pk[:],
                                         AF.Identity, bias=bk_sb[:, m:m + 1], scale=1.0)

            # v[tk, hdv] (+ones col) = y1 @ v_w
            wv = load_weight("v_w")
            bv = bcast_tile("v_b")
            nc.vector.memset(v_ext[:, :, :, DV:], 1.0)
            for m in range(KD):
                for nt2 in range(2):
                    pv = psmm.tile([128, 512], F32, tag="ps_mm", name=pfx + f"pv{m}_{nt2}")
                    for kc in range(KD):
                        nc.tensor.matmul(pv[:], y1T[:, kc, m * 128:(m + 1) * 128],
                                         wv[:, kc, nt2 * 512:(nt2 + 1) * 512],
                                         start=(kc == 0), stop=(kc == KD - 1))
                    nc.vector.tensor_tensor(
                        v_ext[:, m, nt2 * 8:(nt2 + 1) * 8, :DV],
                        pv.rearrange("p (h v) -> p h v", v=DV),
                        bv[:, nt2 * 512:(nt2 + 1) * 512].rearrange(
                            "p (h v) -> p h v", v=DV),
                        OP.add)

        if stop_after == "C":
            return
        # ---------------- phase D: attention (head pairs) ----------------
        with tc.tile_pool(name=pfx + "att", bufs=2) as att:
            for j in range(KD):      # head pair j -> heads 2j (rows 0:64), 2j+1 (64:128)
                e_sb = att.tile([128, 2, KD, TQC], BF, tag="e_sb", name=pfx + f"e{j}")
                for mt in range(KD):
                    ps0 = psmm.tile([128, TQC], F32, tag="ps_mm", name=pfx + f"s{j}_{mt}a")
                    ps1 = psmm.tile([128, TQC], F32, tag="ps_mm", name=pfx + f"s{j}_{mt}b")
                    # row-tiled pair: K=64 each, concurrent on PE row groups
                    nc.tensor.matmul(ps0[:], kT[0:64, j, mt * 128:(mt + 1) * 128],
                                     qT[0:64, j, :], start=True, stop=True)
                    nc.tensor.matmul(ps1[:], kT[64:128, j, mt * 128:(mt + 1) * 128],
                                     qT[64:128, j, :], start=True, stop=True)
                    nc.scalar.activation(e_sb[:, 0, mt, :], ps0[:], AF.Exp,
                                         bias=mb_sb[:, mt:mt + 1], scale=0.125)
                    nc.scalar.activation(e_sb[:, 1, mt, :], ps1[:], AF.Exp,
                                         bias=mb_sb[:, mt:mt + 1], scale=0.125)
                for par in range(2):
                    h = 2 * j + par
                    oh = par * 64
                    ps_av = psmi.tile([128, TQC], F32, tag="ps_av", name=pfx + f"av{h}")
                    for kt in range(KD):
                        nc.tensor.matmul(ps_av[:DV + 1, :], v_ext[:, kt, h, :],
                                         e_sb[:, par, kt, :],
                                         start=(kt == 0), stop=(kt == KD - 1))
                    rcp = att.tile([128, TQC], F32, tag="rcp", name=pfx + f"rc{h}")
                    nc.vector.reciprocal(rcp[DV:DV + 1, :], ps_av[DV:DV + 1, :])
                    rcb = att.tile([128, TQC], BF, tag="rcb", name=pfx + f"rb{h}")
                    nc.vector.tensor_copy(rcb[DV:DV + 1, :], rcp[DV:DV + 1, :])
                    ps_bc = psmi.tile([DV, TQC], F32, tag="ps_tr", name=pfx + f"bc{h}")
                    nc.tensor.matmul(ps_bc[:], ones_c[DV:DV + 1, :],
                                     rcb[DV:DV + 1, :], start=True, stop=True)
                    rb_sb = att.tile([DV, TQC], F32, tag="rb_sb", name=pfx + f"rs{h}")
                    nc.scalar.activation(rb_sb[:], ps_bc[:], AF.Identity, scale=1.0)
                    nc.vector.tensor_tensor(attnT[oh:oh + DV, j, :], ps_av[:DV, :],
                                            rb_sb[:], OP.mult)

        if stop_after == "D":
            return
        # ---------------- phase E: o-proj + residual ----------------
        wo = load_weight("o_w")
        bo = bcast_tile("o_b")
        for mt in range(NT):
            for nt2 in range(2):
                po = psmm.tile([128, 512], F32, tag="ps_mm", name=pfx + f"po{mt}_{nt2}")
                for kc in range(KD):
                    nc.tensor.matmul(po[:], attnT[:, kc, mt * 128:(mt + 1) * 128],
                                     wo[:, kc, nt2 * 512:(nt2 + 1) * 512],
                                     start=(kc == 0), stop=(kc == KD - 1))
                sl = slice(nt2 * 512, (nt2 + 1) * 512)
                nc.vector.tensor_tensor(x_sb[:, mt, sl], x_sb[:, mt, sl], po[:], OP.add)
                nc.gpsimd.tensor_tensor(x_sb[:, mt, sl], x_sb[:, mt, sl], bo[:, sl], OP.add)

        if stop_after == "E":
            return
        # ---------------- phases F/G: MLP ----------------
        with (
            tc.tile_pool(name=pfx + "mlp", bufs=1) as mlp,
            tc.tile_pool(name=pfx + "mtr", bufs=1) as mtr,
        ):
            g3 = bcast_tile("mln1_g")
            b3 = bcast_tile("mln1_b")
            z1 = mlp.tile([128, NT, D], BF, tag="z1", name=pfx + "z1")
            for t in range(NT):
                ln_relu(x_sb[:, t, :], g3, b3, z1[:, t, :], f"z1_{t}")
            z1T = mtr.tile([128, KD, TQC], BF, tag="z1T", name=pfx + "z1T")
            for dt in range(KD):
                for tt in range(NT):
                    transpose_128(z1[:, tt, dt * 128:(dt + 1) * 128],
                                  z1T[:, dt, tt * 128:(tt + 1) * 128])
            w1 = load_weight("l1_w")
            bl1 = bcast_tile("l1_b")
            h_sb = mlp.tile([128, NT, D], F32, tag="h_sb", name=pfx + "h_sb")
            for mt in range(NT):
                for nt2 in range(2):
                    ph = psmm.tile([128, 512], F32, tag="ps_mm", name=pfx + f"ph{mt}_{nt2}")
                    for kc in range(KD):
                        nc.tensor.matmul(ph[:], z1T[:, kc, mt * 128:(mt + 1) * 128],
                                         w1[:, kc, nt2 * 512:(nt2 + 1) * 512],
                                         start=(kc == 0), stop=(kc == KD - 1))
                    sl = slice(nt2 * 512, (nt2 + 1) * 512)
                    nc.vector.tensor_tensor(h_sb[:, mt, sl], ph[:], bl1[:, sl], OP.add)

            g4 = bcast_tile("mln2_g")
            b4 = bcast_tile("mln2_b")
            z2 = mlp.tile([128, NT, D], BF, tag="z2", name=pfx + "z2")
            for t in range(NT):
                ln_relu(h_sb[:, t, :], g4, b4, z2[:, t, :], f"z2_{t}")
            z2T = mtr.tile([128, KD, TQC], BF, tag="z2T", name=pfx + "z2T")
            for dt in range(KD):
                for tt in range(NT):
                    transpose_128(z2[:, tt, dt * 128:(dt + 1) * 128],
                                  z2T[:, dt, tt * 128:(tt + 1) * 128])
            w2 = load_weight("l2_w")
            bl2 = bcast_tile("l2_b")
            out_r = out.rearrange("(t p) d -> p t d", p=128)
            for mt in range(NT):
                o_sb = mlp.tile([128, D], F32, tag="o_sb", name=pfx + f"os{mt}")
                for nt2 in range(2):
                    pf = psmm.tile([128, 512], F32, tag="ps_mm", name=pfx + f"pf{mt}_{nt2}")
                    for kc in range(KD):
                        nc.tensor.matmul(pf[:], z2T[:, kc, mt * 128:(mt + 1) * 128],
                                         w2[:, kc, nt2 * 512:(nt2 + 1) * 512],
                                         start=(kc == 0), stop=(kc == KD - 1))
                    sl = slice(nt2 * 512, (nt2 + 1) * 512)
                    nc.vector.tensor_tensor(o_sb[:, sl], pf[:], bl2[:, sl], OP.add)
                nc.sync.dma_start(out_r[:, mt, :], o_sb[:])


_NC_CACHE = None


def _get_nc():
    global _NC_CACHE
    if _NC_CACHE is None:
        _NC_CACHE = build_kernel()
    return _NC_CACHE


def make_in_maps(inputs):
    """Split full inputs into 8 per-core input maps."""
    x = np.asarray(inputs["x"], np.float32)
    y = np.asarray(inputs["y"], np.float32)
    mask = np.asarray(inputs["mask"])
    shared = {}
    for n in WEIGHT_NAMES:
        shared[n] = np.ascontiguousarray(
            np.asarray(inputs[n], np.float32).astype(ml_dtypes.bfloat16))
    for n in VEC_NAMES:
        shared[n] = np.ascontiguousarray(np.asarray(inputs[n], np.float32))
    in_maps = []
    for c in range(8):
        b, qh = c // 2, c % 2
        m = dict(shared)
        m["xs"] = np.ascontiguousarray(x[b, qh * TQC:(qh + 1) * TQC, :])
        m["y"] = np.ascontiguousarray(y[b])
        m["mb"] = ((mask[b].astype(np.float32) - 1.0) * -MASK_NEG).astype(np.float32)
        in_maps.append(m)
    return in_maps


def assemble(results):
    outf = np.empty((B, TQ, D), np.float32)
    for c in range(8):
        b, qh = c // 2, c % 2
        outf[b, qh * TQC:(qh + 1) * TQC, :] = results[c]["out"]
    return outf


def kernel(**inputs) -> np.ndarray:
    nc = _get_nc()
    in_maps = make_in_maps(inputs)
    res = run_bass_kernel_spmd(nc, in_maps, list(range(8)))
    return assemble(res.results)


if __name__ == "__main__":
    nc = _get_nc()
    print("kernel built and compiled OK")



# revision 2
# speedup vs baseline: 8.3374x; 8.3374x over previous
"""CrossTransformer Trainium2 kernel, v2.

Shapes (hardcoded): B=4, TQ=TK=1024, D=1024, H=16, DK=DV=64.
Sharding: 8 cores = 4 batches x 2 query-row halves. Each core computes
out[b, qs:qs+512, :] independently (k/v work duplicated across the pair
of cores sharing a batch; no collectives).

v2 changes vs v1:
 - mask folded into V multiplicatively (masked tk rows of V and the
   ones-denominator column zeroed once) so the attention exp needs no
   per-tile bias and can run as large [128, 4x512] ACT ops straight
   from bf16 PSUM logits.
 - transposes batched 4-per-PSUM-bank with a single strided copy out.
 - LN affine runs on GpSimd in bf16; ReLU on ScalarE.
 - weight DMAs triple-buffered so they prefetch under compute.
 - optional For_i hardware loop (timing NEFFs run the body R times per
   dispatch to amortize host dispatch overhead).
"""
import sys

for _p in ("/root/.axon_site", "/root/.axon_site/_ro/trn_rl_repo",
           "/root/.axon_site/_ro/pypackages", "/opt/trn_rl_repo"):
    if _p not in sys.path:
        sys.path.append(_p)

import numpy as np
import ml_dtypes
import concourse.bass as bass
from concourse import bacc
import concourse.tile as tile
import concourse.mybir as mybir
from concourse.masks import make_identity
from concourse.bass_utils import run_bass_kernel_spmd

F32 = mybir.dt.float32
BF = mybir.dt.bfloat16
AF = mybir.ActivationFunctionType
OP = mybir.AluOpType

B, TQ, TK, D = 4, 1024, 1024, 1024
H, DK, DV = 16, 64, 64
TQC = TQ // 2          # 512 query rows per core
NT = TQC // 128        # 4 q-row tiles
KD = D // 128          # 8 contraction chunks

WEIGHT_NAMES = ["q_w", "k_w", "v_w", "o_w", "l1_w", "l2_w"]
VEC_NAMES = ["q_b", "k_b", "v_b", "o_b", "l1_b", "l2_b",
             "ln1_g", "ln1_b", "ln2_g", "ln2_b",
             "mln1_g", "mln1_b", "mln2_g", "mln2_b"]


def build_kernel(compile=True, loop=0, repeat=1):
    nc = bacc.Bacc()
    xs = nc.dram_tensor("xs", (TQC, D), F32, kind="ExternalInput")
    y = nc.dram_tensor("y", (TK, D), F32, kind="ExternalInput")
    mb = nc.dram_tensor("mb", (TK,), F32, kind="ExternalInput")
    wd = {n: nc.dram_tensor(n, (D, D), BF, kind="ExternalInput") for n in WEIGHT_NAMES}
    vd = {n: nc.dram_tensor(n, (D,), F32, kind="ExternalInput") for n in VEC_NAMES}
    out = nc.dram_tensor("out", (TQC, D), F32, kind="ExternalOutput")

    with tile.TileContext(nc) as tc:
        if loop:
            with tc.For_i(0, loop):
                _emit(nc, tc, xs, y, mb, wd, vd, out)
        else:
            for r in range(repeat):
                _emit(nc, tc, xs, y, mb, wd, vd, out,
                      pfx=f"r{r}_" if repeat > 1 else "")
    if compile:
        nc.compile()
    return nc


def _emit(nc, tc, xs, y, mb, wd, vd, out, pfx=""):
    from contextlib import ExitStack

    ctx = ExitStack()
    with ctx:
        persist = ctx.enter_context(tc.tile_pool(name=pfx + "persist", bufs=1))
        lnp = ctx.enter_context(tc.tile_pool(name=pfx + "lnp", bufs=2))
        bcast = ctx.enter_context(tc.tile_pool(name=pfx + "bcast", bufs=4))
        wts = ctx.enter_context(tc.tile_pool(name=pfx + "wts", bufs=3))

        # ---------------- constants ----------------
        ident = persist.tile([128, 128], BF, tag="ident", name=pfx + "ident")
        make_identity(nc, ident)
        eps_t = persist.tile([128, 1], F32, tag="eps", name=pfx + "eps")
        nc.vector.memset(eps_t[:], 1e-5)
        ones_c = persist.tile([128, DV], BF, tag="ones_c", name=pfx + "ones_c")
        nc.vector.memset(ones_c[:], 1.0)
        # mask bits (1.0 keep / 0.0 drop) laid out [tk%128, tkblock]
        mbs = persist.tile([128, KD], F32, tag="mbs", name=pfx + "mbs")
        nc.sync.dma_start(mbs[:], mb.rearrange("(t p) -> p t", p=128))
        bq_sb = persist.tile([128, KD], F32, tag="bq_sb", name=pfx + "bq_sb")
        nc.sync.dma_start(bq_sb[:], vd["q_b"].rearrange("(t p) -> p t", p=128))
        bk_sb = persist.tile([128, KD], F32, tag="bk_sb", name=pfx + "bk_sb")
        nc.sync.dma_start(bk_sb[:], vd["k_b"].rearrange("(t p) -> p t", p=128))

        def bcast_tile(name, dt=F32):
            t = bcast.tile([128, D], dt, tag="bc", name=pfx + f"bc_{name}")
            eng = nc.gpsimd if dt != F32 else nc.sync
            eng.dma_start(t[:], vd[name][:].unsqueeze(0).partition_broadcast(128))
            return t

        def load_weight(name):
            wt = wts.tile([128, KD, D], BF, tag="wbf", name=pfx + f"wbf_{name}")
            nc.sync.dma_start(wt[:], wd[name].rearrange("(ko p) n -> p ko n", p=128))
            return wt

        # LN(+affine)+ReLU: src [128, D] f32 -> dst [128, D] bf16
        def ln_relu(src, gt, bt, dst, key):
            stats = lnp.tile([128, 2, 6], F32, tag="stats", name=pfx + f"st_{key}")
            for i in range(2):
                nc.vector.bn_stats(stats[:, i, :], src[:, i * 512:(i + 1) * 512])
            mv = lnp.tile([128, 2], F32, tag="mv", name=pfx + f"mv_{key}")
            nc.vector.bn_aggr(mv[:], stats[:])
            std = lnp.tile([128, 1], F32, tag="std", name=pfx + f"sd_{key}")
            nc.scalar.activation(std[:], mv[:, 1:2], AF.Sqrt, bias=eps_t[:], scale=1.0)
            rstd = lnp.tile([128, 1], F32, tag="rstd", name=pfx + f"rs_{key}")
            nc.vector.reciprocal(rstd[:], std[:])
            z = lnp.tile([128, D], BF, tag="lnz", name=pfx + f"z_{key}")
            nc.vector.tensor_scalar(z[:], src[:], mv[:, 0:1], rstd[:],
                                    OP.subtract, OP.mult)
            nc.gpsimd.tensor_tensor(z[:], z[:], gt[:], OP.mult)
            nc.gpsimd.tensor_tensor(z[:], z[:], bt[:], OP.add)
            nc.scalar.activation(dst, z[:], AF.Relu, scale=1.0)

        # transpose 4 [128,128] bf16 blocks through one PSUM bank, one copy out
        def transpose_quad(pstr, srcs, dst_ap, key):
            pt = pstr.tile([128, 4, 128], BF, tag="ps_tr", name=pfx + f"tr_{key}")
            for k, s in enumerate(srcs):
                nc.tensor.transpose(pt[:, k, :], s, ident[:])
            nc.vector.tensor_copy(dst_ap, pt[:])

        x_sb = persist.tile([128, NT, D], F32, tag="x_sb", name=pfx + "x_sb")
        qT = persist.tile([128, KD, TQC], BF, tag="qT", name=pfx + "qT")
        kT = persist.tile([128, KD, TK], BF, tag="kT", name=pfx + "kT")
        v_ext = persist.tile([128, KD, H, DV + 1], BF, tag="v_ext", name=pfx + "v_ext")
        attnT = persist.tile([128, KD, TQC], BF, tag="attnT", name=pfx + "attnT")

        with (
            tc.tile_pool(name=pfx + "pab", bufs=3) as pab,
            tc.tile_pool(name=pfx + "ptr", bufs=1) as ptr,
            tc.tile_pool(name=pfx + "pstr", bufs=2, space="PSUM") as pstr,
            tc.tile_pool(name=pfx + "psmC", bufs=4, space="PSUM") as psmm,
        ):
            # -------- phase A: LN + relu + transpose (x side, then y side)
            g1 = bcast_tile("ln1_g", BF)
            b1 = bcast_tile("ln1_b", BF)
            g2 = bcast_tile("ln2_g", BF)
            b2 = bcast_tile("ln2_b", BF)

            x1T = ptr.tile([128, KD, TQC], BF, tag="x1T", name=pfx + "x1T")
            for t in range(NT):
                nc.sync.dma_start(x_sb[:, t, :],
                                  xs.rearrange("(t p) d -> p t d", p=128)[:, t, :])
                xz = pab.tile([128, D], BF, tag="xz", name=pfx + f"xz{t}")
                ln_relu(x_sb[:, t, :], g1, b1, xz[:], f"x{t}")
                for g in range(2):
                    transpose_quad(
                        pstr,
                        [xz[:, (g * 4 + k) * 128:(g * 4 + k + 1) * 128]
                         for k in range(4)],
                        x1T[:, g * 4:(g + 1) * 4, t * 128:(t + 1) * 128],
                        f"x{t}_{g}")

            y1T = ptr.tile([128, KD, TK], BF, tag="y1T", name=pfx + "y1T")
            for t in range(KD):
                yl = pab.tile([128, D], F32, tag="yload", name=pfx + f"yl_{t}")
                nc.sync.dma_start(yl[:], y.rearrange("(t p) d -> p t d", p=128)[:, t, :])
                yz = pab.tile([128, D], BF, tag="yz", name=pfx + f"yz{t}")
                ln_relu(yl[:], g2, b2, yz[:], f"y{t}")
                for g in range(2):
                    transpose_quad(
                        pstr,
                        [yz[:, (g * 4 + k) * 128:(g * 4 + k + 1) * 128]
                         for k in range(4)],
                        y1T[:, g * 4:(g + 1) * 4, t * 128:(t + 1) * 128],
                        f"y{t}_{g}")

            # -------- phase C: projections --------
            # qT[hd, tq] = q_w.T @ x1T
            wq = load_weight("q_w")
            for m in range(KD):
                pq = psmm.tile([128, TQC], F32, tag="ps_mm", name=pfx + f"pq{m}")
                for kc in range(KD):
                    nc.tensor.matmul(pq[:], wq[:, kc, m * 128:(m + 1) * 128],
                                     x1T[:, kc, :],
                                     start=(kc == 0), stop=(kc == KD - 1))
                nc.scalar.activation(qT[:, m, :], pq[:], AF.Identity,
                                     bias=bq_sb[:, m:m + 1], scale=1.0)

            # kT[hd, tk] = k_w.T @ y1T
            wk = load_weight("k_w")
            for m in range(KD):
                for nt2 in range(2):
                    pk = psmm.tile([128, 512], F32, tag="ps_mm",
                                   name=pfx + f"pk{m}_{nt2}")
                    for kc in range(KD):
                        nc.tensor.matmul(pk[:], wk[:, kc, m * 128:(m + 1) * 128],
                                         y1T[:, kc, nt2 * 512:(nt2 + 1) * 512],
                                         start=(kc == 0), stop=(kc == KD - 1))
                    nc.scalar.activation(kT[:, m, nt2 * 512:(nt2 + 1) * 512], pk[:],
                                         AF.Identity, bias=bk_sb[:, m:m + 1], scale=1.0)

            # v[tk, hdv] (+mask col) = y1 @ v_w ; then mask rows
            wv = load_weight("v_w")
            bv = bcast_tile("v_b")
            nc.vector.memset(v_ext[:, :, :, DV:], 1.0)
            for m in range(KD):
                for nt2 in range(2):
                    pv = psmm.tile([128, 512], F32, tag="ps_mm",
                                   name=pfx + f"pv{m}_{nt2}")
                    for kc in range(KD):
                        nc.tensor.matmul(pv[:], y1T[:, kc, m * 128:(m + 1) * 128],
                                         wv[:, kc, nt2 * 512:(nt2 + 1) * 512],
                                         start=(kc == 0), stop=(kc == KD - 1))
                    nc.vector.tensor_tensor(
                        v_ext[:, m, nt2 * 8:(nt2 + 1) * 8, :DV],
                        pv.rearrange("p (h v) -> p h v", v=DV),
                        bv[:, nt2 * 512:(nt2 + 1) * 512].rearrange(
                            "p (h v) -> p h v", v=DV),
                        OP.add)
                # zero masked tk rows (incl. the ones column -> denominator)
                nc.gpsimd.tensor_scalar_mul(
                    v_ext[:, m, :, :].rearrange("p h v -> p (h v)"),
                    v_ext[:, m, :, :].rearrange("p h v -> p (h v)"),
                    mbs[:, m:m + 1])

        # -------- phase D: attention (head pairs) --------
        with (
            tc.tile_pool(name=pfx + "att", bufs=2) as att,
            tc.tile_pool(name=pfx + "psL", bufs=3, space="PSUM") as psL,
            tc.tile_pool(name=pfx + "psA", bufs=2, space="PSUM") as psA,
        ):
            for j in range(KD):      # heads 2j (partitions 0:64), 2j+1 (64:128)
                e_sb = att.tile([128, 2, KD, TQC], BF, tag="e_sb", name=pfx + f"e{j}")
                for par in range(2):
                    hp = slice(par * 64, (par + 1) * 64)
                    for ch in range(4):
                        sl = psL.tile([128, 2, 512], F32, tag="ps_l",
                                      name=pfx + f"s{j}_{par}{ch}")
                        for k in range(2):
                            mt = ch * 2 + k
                            nc.tensor.matmul(sl[:, k, :],
                                             kT[hp, j, mt * 128:(mt + 1) * 128],
                                             qT[hp, j, :], start=True, stop=True)
                        nc.scalar.activation(
                            e_sb[:, par, ch * 2:(ch + 1) * 2, :], sl[:],
                            AF.Exp, scale=0.125)
                for par in range(2):
                    h = 2 * j + par
                    oh = par * 64
                    ps_av = psA.tile([128, TQC], F32, tag="ps_av", name=pfx + f"av{h}")
                    for kt in range(KD):
                        nc.tensor.matmul(ps_av[:DV + 1, :], v_ext[:, kt, h, :],
                                         e_sb[:, par, kt, :],
                                         start=(kt == 0), stop=(kt == KD - 1))
                    rcp = att.tile([128, TQC], F32, tag="rcp", name=pfx + f"rc{h}")
                    nc.vector.reciprocal(rcp[DV:DV + 1, :], ps_av[DV:DV + 1, :])
                    rcb = att.tile([128, TQC], BF, tag="rcb", name=pfx + f"rb{h}")
                    nc.vector.tensor_copy(rcb[DV:DV + 1, :], rcp[DV:DV + 1, :])
                    ps_bc = psA.tile([128, TQC], F32, tag="ps_av", name=pfx + f"bc{h}")
                    nc.tensor.matmul(ps_bc[:DV, :], ones_c[DV:DV + 1, :],
                                     rcb[DV:DV + 1, :], start=True, stop=True)
                    rb_sb = att.tile([DV, TQC], BF, tag="rb_sb", name=pfx + f"rs{h}")
                    nc.scalar.activation(rb_sb[:], ps_bc[:DV, :], AF.Identity,
                                         scale=1.0)
                    nc.vector.tensor_tensor(attnT[oh:oh + DV, j, :], ps_av[:DV, :],
                                            rb_sb[:], OP.mult)

        # -------- phase E: o-proj + residual --------
        psmE = ctx.enter_context(tc.tile_pool(name=pfx + "psmE", bufs=4,
                                              space="PSUM"))
        pstr2 = ctx.enter_context(tc.tile_pool(name=pfx + "pstr2", bufs=2,
                                               space="PSUM"))
        psmm = psmE
        wo = load_weight("o_w")
        bo = bcast_tile("o_b")
        for mt in range(NT):
            for nt2 in range(2):
                po = psmm.tile([128, 512], F32, tag="ps_mm", name=pfx + f"po{mt}_{nt2}")
                for kc in range(KD):
                    nc.tensor.matmul(po[:], attnT[:, kc, mt * 128:(mt + 1) * 128],
                                     wo[:, kc, nt2 * 512:(nt2 + 1) * 512],
                                     start=(kc == 0), stop=(kc == KD - 1))
                sl = slice(nt2 * 512, (nt2 + 1) * 512)
                nc.vector.tensor_tensor(x_sb[:, mt, sl], x_sb[:, mt, sl], po[:], OP.add)
                nc.gpsimd.tensor_tensor(x_sb[:, mt, sl], x_sb[:, mt, sl], bo[:, sl],
                                        OP.add)

        # -------- phases F/G: MLP --------
        with (
            tc.tile_pool(name=pfx + "mlp", bufs=1) as mlp,
            tc.tile_pool(name=pfx + "mz", bufs=2) as mz,
            tc.tile_pool(name=pfx + "mtr", bufs=1) as mtr,
        ):
            g3 = bcast_tile("mln1_g", BF)
            b3 = bcast_tile("mln1_b", BF)
            z1T = mtr.tile([128, KD, TQC], BF, tag="z1T", name=pfx + "z1T")
            for t in range(NT):
                z1 = mz.tile([128, D], BF, tag="z1", name=pfx + f"z1_{t}")
                ln_relu(x_sb[:, t, :], g3, b3, z1[:], f"z1_{t}")
                for g in range(2):
                    transpose_quad(
                        pstr2,
                        [z1[:, (g * 4 + k) * 128:(g * 4 + k + 1) * 128]
                         for k in range(4)],
                        z1T[:, g * 4:(g + 1) * 4, t * 128:(t + 1) * 128],
                        f"z1{t}_{g}")
            w1 = load_weight("l1_w")
            bl1 = bcast_tile("l1_b")
            h_sb = mlp.tile([128, NT, D], F32, tag="h_sb", name=pfx + "h_sb")
            for mt in range(NT):
                for nt2 in range(2):
                    ph = psmm.tile([128, 512], F32, tag="ps_mm",
                                   name=pfx + f"ph{mt}_{nt2}")
                    for kc in range(KD):
                        nc.tensor.matmul(ph[:], z1T[:, kc, mt * 128:(mt + 1) * 128],
                                         w1[:, kc, nt2 * 512:(nt2 + 1) * 512],
                                         start=(kc == 0), stop=(kc == KD - 1))
                    sl = slice(nt2 * 512, (nt2 + 1) * 512)
                    nc.vector.tensor_tensor(h_sb[:, mt, sl], ph[:], bl1[:, sl], OP.add)

            g4 = bcast_tile("mln2_g", BF)
            b4 = bcast_tile("mln2_b", BF)
            z2T = mtr.tile([128, KD, TQC], BF, tag="z2T", name=pfx + "z2T")
            for t in range(NT):
                z2 = mz.tile([128, D], BF, tag="z2", name=pfx + f"z2_{t}")
                ln_relu(h_sb[:, t, :], g4, b4, z2[:], f"z2_{t}")
                for g in range(2):
                    transpose_quad(
                        pstr2,
                        [z2[:, (g * 4 + k) * 128:(g * 4 + k + 1) * 128]
                         for k in range(4)],
                        z2T[:, g * 4:(g + 1) * 4, t * 128:(t + 1) * 128],
                        f"z2{t}_{g}")
            w2 = load_weight("l2_w")
            bl2 = bcast_tile("l2_b")
            out_r = out.rearrange("(t p) d -> p t d", p=128)
            for mt in range(NT):
                o_sb = mlp.tile([128, D], F32, tag="o_sb", name=pfx + f"os{mt}")
                for nt2 in range(2):
                    pf = psmm.tile([128, 512], F32, tag="ps_mm",
                                   name=pfx + f"pf{mt}_{nt2}")
                    for kc in range(KD):
                        nc.tensor.matmul(pf[:], z2T[:, kc, mt * 128:(mt + 1) * 128],
                                         w2[:, kc, nt2 * 512:(nt2 + 1) * 512],
                                         start=(kc == 0), stop=(kc == KD - 1))
                    sl = slice(nt2 * 512, (nt2 + 1) * 512)
                    nc.vector.tensor_tensor(o_sb[:, sl], pf[:], bl2[:, sl], OP.add)
                nc.sync.dma_start(out_r[:, mt, :], o_sb[:])


_NC_CACHE = None


def _get_nc():
    global _NC_CACHE
    if _NC_CACHE is None:
        _NC_CACHE = build_kernel()
    return _NC_CACHE


def make_in_maps(inputs):
    """Split full inputs into 8 per-core input maps."""
    x = np.asarray(inputs["x"], np.float32)
    y = np.asarray(inputs["y"], np.float32)
    mask = np.asarray(inputs["mask"])
    shared = {}
    for n in WEIGHT_NAMES:
        shared[n] = np.ascontiguousarray(
            np.asarray(inputs[n], np.float32).astype(ml_dtypes.bfloat16))
    for n in VEC_NAMES:
        shared[n] = np.ascontiguousarray(np.asarray(inputs[n], np.float32))
    in_maps = []
    for c in range(8):
        b, qh = c // 2, c % 2
        m = dict(shared)
        m["xs"] = np.ascontiguousarray(x[b, qh * TQC:(qh + 1) * TQC, :])
        m["y"] = np.ascontiguousarray(y[b])
        m["mb"] = mask[b].astype(np.float32)
        in_maps.append(m)
    return in_maps


def assemble(results):
    outf = np.empty((B, TQ, D), np.float32)
    for c in range(8):
        b, qh = c // 2, c % 2
        outf[b, qh * TQC:(qh + 1) * TQC, :] = results[c]["out"]
    return outf


def kernel(**inputs) -> np.ndarray:
    nc = _get_nc()
    in_maps = make_in_maps(inputs)
    res = run_bass_kernel_spmd(nc, in_maps, list(range(8)))
    return assemble(res.results)


if __name__ == "__main__":
    nc = _get_nc()
    print("kernel built and compiled OK")


# revision 3
# speedup vs baseline: 9.8742x; 1.1843x over previous
"""CrossTransformer Trainium2 kernel, v2.

Shapes (hardcoded): B=4, TQ=TK=1024, D=1024, H=16, DK=DV=64.
Sharding: 8 cores = 4 batches x 2 query-row halves. Each core computes
out[b, qs:qs+512, :] independently (k/v work duplicated across the pair
of cores sharing a batch; no collectives).

v2 changes vs v1:
 - mask folded into V multiplicatively (masked tk rows of V and the
   ones-denominator column zeroed once) so the attention exp needs no
   per-tile bias and can run as large [128, 4x512] ACT ops straight
   from bf16 PSUM logits.
 - transposes batched 4-per-PSUM-bank with a single strided copy out.
 - LN affine runs on GpSimd in bf16; ReLU on ScalarE.
 - weight DMAs triple-buffered so they prefetch under compute.
 - optional For_i hardware loop (timing NEFFs run the body R times per
   dispatch to amortize host dispatch overhead).
"""
import sys

for _p in ("/root/.axon_site", "/root/.axon_site/_ro/trn_rl_repo",
           "/root/.axon_site/_ro/pypackages", "/opt/trn_rl_repo"):
    if _p not in sys.path:
        sys.path.append(_p)

import os
import numpy as np
import ml_dtypes
import concourse.bass as bass
from concourse import bacc
import concourse.tile as tile
import concourse.mybir as mybir
from concourse.masks import make_identity
from concourse.bass_utils import run_bass_kernel_spmd

F32 = mybir.dt.float32
BF = mybir.dt.bfloat16
AF = mybir.ActivationFunctionType
OP = mybir.AluOpType

RECIP_FAST = int(os.environ.get("K2_RECIP_FAST", "0"))
CAST_DMA = int(os.environ.get("K2_CAST_DMA", "1"))
AFFINE_DVE = int(os.environ.get("K2_AFFINE_DVE", "0"))

B, TQ, TK, D = 4, 1024, 1024, 1024
H, DK, DV = 16, 64, 64
TQC = TQ // 2          # 512 query rows per core
NT = TQC // 128        # 4 q-row tiles
KD = D // 128          # 8 contraction chunks

WEIGHT_NAMES = ["q_w", "k_w", "v_w", "o_w", "l1_w", "l2_w"]
BCAST_NAMES = ["v_b", "o_b", "l1_b", "l2_b",
               "ln1_g", "ln1_b", "ln2_g", "ln2_b",
               "mln1_g", "mln1_b", "mln2_g", "mln2_b"]


def build_kernel(compile=True, loop=0, repeat=1, stop_after=None):
    nc = bacc.Bacc()
    # host pre-arranges everything partition-major and packs the 23 inputs
    # into 3 tensors (each extra NEFF arg costs host dispatch time):
    #   data  [128, 12*D + 3*KD] f32: x tiles, y tiles, mask col, qb col, kb col
    #   wpack [128, 6, KD, D]   bf16: q,k,v,o,l1,l2 weights
    #   vpack [12, D]            f32: bias/gain vectors for broadcast
    data = nc.dram_tensor("data", (128, 12 * D + 3 * KD), F32,
                          kind="ExternalInput")
    wpack = nc.dram_tensor("wpack", (128, 6, KD, D), BF, kind="ExternalInput")
    vpack = nc.dram_tensor("vpack", (len(BCAST_NAMES), D), F32,
                           kind="ExternalInput")
    out = nc.dram_tensor("out", (128, NT, D), F32, kind="ExternalOutput")

    with tile.TileContext(nc) as tc:
        if loop:
            with tc.For_i(0, loop):
                _emit(nc, tc, data, wpack, vpack, out)
        else:
            for r in range(repeat):
                _emit(nc, tc, data, wpack, vpack, out,
                      pfx=f"r{r}_" if repeat > 1 else "",
                      stop_after=stop_after)
    if compile:
        nc.compile()
    return nc


def _emit(nc, tc, data, wpack, vpack, out, pfx="", stop_after=None):
    from contextlib import ExitStack

    ctx = ExitStack()
    with ctx:
        persist = ctx.enter_context(tc.tile_pool(name=pfx + "persist", bufs=1))
        lnp = ctx.enter_context(tc.tile_pool(name=pfx + "lnp", bufs=2))
        bcast = ctx.enter_context(tc.tile_pool(name=pfx + "bcast", bufs=4))
        wts = ctx.enter_context(tc.tile_pool(name=pfx + "wts", bufs=2))

        # ---------------- constants ----------------
        ident = persist.tile([128, 128], BF, tag="ident", name=pfx + "ident")
        make_identity(nc, ident)
        eps_t = persist.tile([128, 1], F32, tag="eps", name=pfx + "eps")
        nc.vector.memset(eps_t[:], 1e-5)
        ones_c = persist.tile([128, DV], BF, tag="ones_c", name=pfx + "ones_c")
        nc.vector.memset(ones_c[:], 1.0)
        # mask bits (1.0 keep / 0.0 drop) laid out [tk%128, tkblock]
        mbs = persist.tile([128, 3 * KD], F32, tag="mbs", name=pfx + "mbs")
        nc.sync.dma_start(mbs[:], data[:, 12 * D:])
        bq_sb = mbs[:, KD:2 * KD]
        bk_sb = mbs[:, 2 * KD:3 * KD]

        def bcast_tile(name, dt=F32):
            if not CAST_DMA:
                dt = F32
            i = BCAST_NAMES.index(name)
            t = bcast.tile([128, D], dt, tag="bc", name=pfx + f"bc_{name}")
            nc.gpsimd.dma_start(t[:],
                                vpack[i:i + 1, :].partition_broadcast(128))
            return t

        def load_weight(name):
            i = WEIGHT_NAMES.index(name)
            wt = wts.tile([128, KD, D], BF, tag="wbf", name=pfx + f"wbf_{name}")
            nc.sync.dma_start(wt[:], wpack[:, i, :, :])
            return wt

        # LN(+affine)+ReLU: src [128, D] f32 -> dst [128, D] bf16
        def ln_relu(src, gt, bt, dst, key, dve=False):
            stats = lnp.tile([128, 2, 6], F32, tag="stats", name=pfx + f"st_{key}")
            for i in range(2):
                nc.vector.bn_stats(stats[:, i, :], src[:, i * 512:(i + 1) * 512])
            mv = lnp.tile([128, 2], F32, tag="mv", name=pfx + f"mv_{key}")
            nc.vector.bn_aggr(mv[:], stats[:])
            std = lnp.tile([128, 1], F32, tag="std", name=pfx + f"sd_{key}")
            nc.scalar.activation(std[:], mv[:, 1:2], AF.Sqrt, bias=eps_t[:], scale=1.0)
            rstd = lnp.tile([128, 1], F32, tag="rstd", name=pfx + f"rs_{key}")
            nc.vector.reciprocal(rstd[:], std[:])
            z = lnp.tile([128, D], BF, tag="lnz", name=pfx + f"z_{key}")
            nc.vector.tensor_scalar(z[:], src[:], mv[:, 0:1], rstd[:],
                                    OP.subtract, OP.mult)
            if dve and AFFINE_DVE:
                nc.vector.tensor_tensor(z[:], z[:], gt[:], OP.mult)
                nc.vector.tensor_tensor(z[:], z[:], bt[:], OP.add)
                nc.vector.tensor_scalar_max(dst, z[:], 0.0)
            else:
                nc.gpsimd.tensor_tensor(z[:], z[:], gt[:], OP.mult)
                nc.gpsimd.tensor_tensor(z[:], z[:], bt[:], OP.add)
                nc.scalar.activation(dst, z[:], AF.Relu, scale=1.0)

        # transpose 4 [128,128] bf16 blocks through one PSUM bank, one copy out
        def transpose_quad(pstr, srcs, dst_ap, key):
            pt = pstr.tile([128, 4, 128], BF, tag="ps_tr", name=pfx + f"tr_{key}")
            for k, s in enumerate(srcs):
                nc.tensor.transpose(pt[:, k, :], s, ident[:])
            nc.vector.tensor_copy(dst_ap, pt[:])

        x_sb = persist.tile([128, NT, D], F32, tag="x_sb", name=pfx + "x_sb")
        qT = persist.tile([128, KD, TQC], BF, tag="qT", name=pfx + "qT")
        kT = persist.tile([128, KD, TK], BF, tag="kT", name=pfx + "kT")
        v_ext = persist.tile([128, KD, H, DV + 1], BF, tag="v_ext", name=pfx + "v_ext")
        attnT = persist.tile([128, KD, TQC], BF, tag="attnT", name=pfx + "attnT")

        with (
            tc.tile_pool(name=pfx + "pab", bufs=3) as pab,
            tc.tile_pool(name=pfx + "ptr", bufs=1) as ptr,
            tc.tile_pool(name=pfx + "pstr", bufs=2, space="PSUM") as pstr,
            tc.tile_pool(name=pfx + "psmC", bufs=4, space="PSUM") as psmm,
        ):
            # -------- phase A: LN + relu + transpose (x side, then y side)
            g1 = bcast_tile("ln1_g", BF)
            b1 = bcast_tile("ln1_b", BF)
            g2 = bcast_tile("ln2_g", BF)
            b2 = bcast_tile("ln2_b", BF)

            x1T = ptr.tile([128, KD, TQC], BF, tag="x1T", name=pfx + "x1T")
            for t in range(NT):
                nc.sync.dma_start(x_sb[:, t, :], data[:, t * D:(t + 1) * D])
                xz = pab.tile([128, D], BF, tag="xz", name=pfx + f"xz{t}")
                ln_relu(x_sb[:, t, :], g1, b1, xz[:], f"x{t}")
                for g in range(2):
                    transpose_quad(
                        pstr,
                        [xz[:, (g * 4 + k) * 128:(g * 4 + k + 1) * 128]
                         for k in range(4)],
                        x1T[:, g * 4:(g + 1) * 4, t * 128:(t + 1) * 128],
                        f"x{t}_{g}")

            y1T = ptr.tile([128, KD, TK], BF, tag="y1T", name=pfx + "y1T")
            for t in range(KD):
                yl = pab.tile([128, D], F32, tag="yload", name=pfx + f"yl_{t}")
                nc.scalar.dma_start(yl[:], data[:, (NT + t) * D:(NT + t + 1) * D])
                yz = pab.tile([128, D], BF, tag="yz", name=pfx + f"yz{t}")
                ln_relu(yl[:], g2, b2, yz[:], f"y{t}")
                for g in range(2):
                    transpose_quad(
                        pstr,
                        [yz[:, (g * 4 + k) * 128:(g * 4 + k + 1) * 128]
                         for k in range(4)],
                        y1T[:, g * 4:(g + 1) * 4, t * 128:(t + 1) * 128],
                        f"y{t}_{g}")

            if stop_after == "A":
                return
            # -------- phase C: projections --------
            # qT[hd, tq] = q_w.T @ x1T
            wq = load_weight("q_w")
            for m in range(KD):
                pq = psmm.tile([128, TQC], F32, tag="ps_mm", name=pfx + f"pq{m}")
                for kc in range(KD):
                    nc.tensor.matmul(pq[:], wq[:, kc, m * 128:(m + 1) * 128],
                                     x1T[:, kc, :],
                                     start=(kc == 0), stop=(kc == KD - 1))
                nc.scalar.activation(qT[:, m, :], pq[:], AF.Identity,
                                     bias=mbs[:, KD + m:KD + m + 1], scale=1.0)

            # kT[hd, tk] = k_w.T @ y1T
            wk = load_weight("k_w")
            for m in range(KD):
                for nt2 in range(2):
                    pk = psmm.tile([128, 512], F32, tag="ps_mm",
                                   name=pfx + f"pk{m}_{nt2}")
                    for kc in range(KD):
                        nc.tensor.matmul(pk[:], wk[:, kc, m * 128:(m + 1) * 128],
                                         y1T[:, kc, nt2 * 512:(nt2 + 1) * 512],
                                         start=(kc == 0), stop=(kc == KD - 1))
                    nc.scalar.activation(kT[:, m, nt2 * 512:(nt2 + 1) * 512], pk[:],
                                         AF.Identity, bias=mbs[:, 2 * KD + m:2 * KD + m + 1], scale=1.0)

            # v[tk, hdv] (+mask col) = y1 @ v_w ; then mask rows
            wv = load_weight("v_w")
            bv = bcast_tile("v_b")
            nc.vector.memset(v_ext[:, :, :, DV:], 1.0)
            for m in range(KD):
                for nt2 in range(2):
                    pv = psmm.tile([128, 512], F32, tag="ps_mm",
                                   name=pfx + f"pv{m}_{nt2}")
                    for kc in range(KD):
                        nc.tensor.matmul(pv[:], y1T[:, kc, m * 128:(m + 1) * 128],
                                         wv[:, kc, nt2 * 512:(nt2 + 1) * 512],
                                         start=(kc == 0), stop=(kc == KD - 1))
                    nc.vector.tensor_tensor(
                        v_ext[:, m, nt2 * 8:(nt2 + 1) * 8, :DV],
                        pv.rearrange("p (h v) -> p h v", v=DV),
                        bv[:, nt2 * 512:(nt2 + 1) * 512].rearrange(
                            "p (h v) -> p h v", v=DV),
                        OP.add)
                # zero masked tk rows (incl. the ones column -> denominator)
                nc.gpsimd.tensor_scalar_mul(
                    v_ext[:, m, :, :].rearrange("p h v -> p (h v)"),
                    v_ext[:, m, :, :].rearrange("p h v -> p (h v)"),
                    mbs[:, m:m + 1])

        if stop_after == "C":
            return
        # -------- phase D: attention (head pairs) --------
        # Rescale is batched: av results land in SBUF, the 8 denominator
        # rows of a 4-pair batch are DMA-gathered onto adjacent partitions
        # and divided with ONE reciprocal (HW reciprocal is ~8 cyc/elem,
        # so 16 separate [1,512] divides cost ~100us/iter).
        with (
            tc.tile_pool(name=pfx + "att", bufs=2) as att,
            tc.tile_pool(name=pfx + "psL", bufs=2, space="PSUM") as psL,
            tc.tile_pool(name=pfx + "psA", bufs=2, space="PSUM") as psA,
            tc.tile_pool(name=pfx + "psB", bufs=2, space="PSUM") as psB,
        ):
            # sel8[:, h, :] is a [8, DV] one-hot-row selector: row h is ones
            sel8 = persist.tile([8, 8, DV], BF, tag="sel8", name=pfx + "sel8")
            nc.vector.memset(sel8[:], 0.0)
            for k in range(8):
                nc.gpsimd.dma_start(sel8[k:k + 1, k, :], ones_c[0:1, :DV])
            for half in range(2):
                av_all = att.tile([128, 8, TQC], F32, tag="av_all",
                                  name=pfx + f"avh{half}", bufs=1)
                den8 = att.tile([8, TQC], F32, tag="den8", name=pfx + f"dn{half}")
                for jj in range(4):
                    j = half * 4 + jj
                    e_sb = att.tile([128, 2, KD, TQC], BF, tag="e_sb",
                                    name=pfx + f"e{j}")
                    for par in range(2):
                        hp = slice(par * 64, (par + 1) * 64)
                        for ch in range(4):
                            sl = psL.tile([128, 2, 512], F32, tag="ps_l",
                                          name=pfx + f"s{j}_{par}{ch}")
                            for k in range(2):
                                mt = ch * 2 + k
                                nc.tensor.matmul(sl[:, k, :],
                                                 kT[hp, j, mt * 128:(mt + 1) * 128],
                                                 qT[hp, j, :], start=True, stop=True)
                            nc.scalar.activation(
                                e_sb[:, par, ch * 2:(ch + 1) * 2, :], sl[:],
                                AF.Exp, scale=0.125)
                    if stop_after == "D1":
                        continue
                    for par in range(2):
                        h = 2 * j + par
                        hloc = 2 * jj + par
                        ps_av = psA.tile([128, TQC], F32, tag="ps_av",
                                         name=pfx + f"av{h}")
                        for kt in range(KD):
                            nc.tensor.matmul(ps_av[:DV + 1, :], v_ext[:, kt, h, :],
                                             e_sb[:, par, kt, :],
                                             start=(kt == 0), stop=(kt == KD - 1))
                        if stop_after == "D2":
                            nc.vector.tensor_copy(attnT[par * 64:par * 64 + DV, j, :],
                                                  ps_av[:DV, :])
                            continue
                        nc.vector.tensor_copy(av_all[:DV + 1, hloc, :],
                                              ps_av[:DV + 1, :])
                        nc.gpsimd.dma_start(den8[hloc:hloc + 1, :],
                                            av_all[DV:DV + 1, hloc, :])
                if stop_after in ("D1", "D2"):
                    continue
                rcp8 = att.tile([8, TQC], F32, tag="rcp8", name=pfx + f"rp{half}")
                nc.vector.reciprocal(rcp8[:], den8[:])
                rcb8 = att.tile([8, TQC], BF, tag="rcb8", name=pfx + f"rb{half}")
                nc.vector.tensor_copy(rcb8[:], rcp8[:])
                for jj in range(4):
                    j = half * 4 + jj
                    for par in range(2):
                        hloc = 2 * jj + par
                        oh = par * 64
                        ps_bc = psB.tile([DV, TQC], F32, tag="ps_bc",
                                         name=pfx + f"bc{half}_{hloc}")
                        nc.tensor.matmul(ps_bc[:], sel8[:, hloc, :], rcb8[:],
                                         start=True, stop=True)
                        nc.vector.tensor_tensor(attnT[oh:oh + DV, j, :],
                                                av_all[:DV, hloc, :],
                                                ps_bc[:], OP.mult)

        if stop_after in ("D", "D1", "D2"):
            return
        # -------- phase E: o-proj + residual --------
        psmE = ctx.enter_context(tc.tile_pool(name=pfx + "psmE", bufs=4,
                                              space="PSUM"))
        pstr2 = ctx.enter_context(tc.tile_pool(name=pfx + "pstr2", bufs=2,
                                               space="PSUM"))
        psmm = psmE
        wo = load_weight("o_w")
        bo = bcast_tile("o_b")
        for mt in range(NT):
            for nt2 in range(2):
                po = psmm.tile([128, 512], F32, tag="ps_mm", name=pfx + f"po{mt}_{nt2}")
                for kc in range(KD):
                    nc.tensor.matmul(po[:], attnT[:, kc, mt * 128:(mt + 1) * 128],
                                     wo[:, kc, nt2 * 512:(nt2 + 1) * 512],
                                     start=(kc == 0), stop=(kc == KD - 1))
                sl = slice(nt2 * 512, (nt2 + 1) * 512)
                nc.vector.tensor_tensor(x_sb[:, mt, sl], x_sb[:, mt, sl], po[:], OP.add)
                nc.gpsimd.tensor_tensor(x_sb[:, mt, sl], x_sb[:, mt, sl], bo[:, sl],
                                        OP.add)

        if stop_after == "E":
            return
        # -------- phases F/G: MLP --------
        with (
            tc.tile_pool(name=pfx + "mlp", bufs=1) as mlp,
            tc.tile_pool(name=pfx + "mz", bufs=2) as mz,
            tc.tile_pool(name=pfx + "mtr", bufs=1) as mtr,
        ):
            g3 = bcast_tile("mln1_g", BF)
            b3 = bcast_tile("mln1_b", BF)
            z1T = mtr.tile([128, KD, TQC], BF, tag="z1T", name=pfx + "z1T")
            for t in range(NT):
                z1 = mz.tile([128, D], BF, tag="z1", name=pfx + f"z1_{t}")
                ln_relu(x_sb[:, t, :], g3, b3, z1[:], f"z1_{t}", dve=True)
                for g in range(2):
                    transpose_quad(
                        pstr2,
                        [z1[:, (g * 4 + k) * 128:(g * 4 + k + 1) * 128]
                         for k in range(4)],
                        z1T[:, g * 4:(g + 1) * 4, t * 128:(t + 1) * 128],
                        f"z1{t}_{g}")
            w1 = load_weight("l1_w")
            bl1 = bcast_tile("l1_b")
            h_sb = mlp.tile([128, NT, D], F32, tag="h_sb", name=pfx + "h_sb")
            for mt in range(NT):
                for nt2 in range(2):
                    ph = psmm.tile([128, 512], F32, tag="ps_mm",
                                   name=pfx + f"ph{mt}_{nt2}")
                    for kc in range(KD):
                        nc.tensor.matmul(ph[:], z1T[:, kc, mt * 128:(mt + 1) * 128],
                                         w1[:, kc, nt2 * 512:(nt2 + 1) * 512],
                                         start=(kc == 0), stop=(kc == KD - 1))
                    sl = slice(nt2 * 512, (nt2 + 1) * 512)
                    nc.vector.tensor_tensor(h_sb[:, mt, sl], ph[:], bl1[:, sl], OP.add)

            if stop_after == "F":
                return
            g4 = bcast_tile("mln2_g", BF)
            b4 = bcast_tile("mln2_b", BF)
            z2T = mtr.tile([128, KD, TQC], BF, tag="z2T", name=pfx + "z2T")
            for t in range(NT):
                z2 = mz.tile([128, D], BF, tag="z2", name=pfx + f"z2_{t}")
                ln_relu(h_sb[:, t, :], g4, b4, z2[:], f"z2_{t}", dve=True)
                for g in range(2):
                    transpose_quad(
                        pstr2,
                        [z2[:, (g * 4 + k) * 128:(g * 4 + k + 1) * 128]
                         for k in range(4)],
                        z2T[:, g * 4:(g + 1) * 4, t * 128:(t + 1) * 128],
                        f"z2{t}_{g}")
            w2 = load_weight("l2_w")
            bl2 = bcast_tile("l2_b")
            out_r = out
            for mt in range(NT):
                o_sb = mlp.tile([128, D], F32, tag="o_sb", name=pfx + f"os{mt}")
                for nt2 in range(2):
                    pf = psmm.tile([128, 512], F32, tag="ps_mm",
                                   name=pfx + f"pf{mt}_{nt2}")
                    for kc in range(KD):
                        nc.tensor.matmul(pf[:], z2T[:, kc, mt * 128:(mt + 1) * 128],
                                         w2[:, kc, nt2 * 512:(nt2 + 1) * 512],
                                         start=(kc == 0), stop=(kc == KD - 1))
                    sl = slice(nt2 * 512, (nt2 + 1) * 512)
                    nc.vector.tensor_tensor(o_sb[:, sl], pf[:], bl2[:, sl], OP.add)
                nc.scalar.dma_start(out_r[:, mt, :], o_sb[:])


_NC_CACHE = None


def _get_nc():
    global _NC_CACHE
    if _NC_CACHE is None:
        _NC_CACHE = build_kernel()
    return _NC_CACHE


def make_in_maps(inputs):
    """Split full inputs into 8 per-core input maps (packed, partition-major)."""
    x = np.asarray(inputs["x"], np.float32)
    y = np.asarray(inputs["y"], np.float32)
    mask = np.asarray(inputs["mask"])

    wpack = np.empty((128, 6, KD, D), ml_dtypes.bfloat16)
    for i, n in enumerate(WEIGHT_NAMES):
        w = np.asarray(inputs[n], np.float32).astype(ml_dtypes.bfloat16)
        wpack[:, i] = w.reshape(KD, 128, D).transpose(1, 0, 2)
    vpack = np.stack([np.asarray(inputs[n], np.float32) for n in BCAST_NAMES])
    qb_col = np.asarray(inputs["q_b"], np.float32).reshape(KD, 128).T
    kb_col = np.asarray(inputs["k_b"], np.float32).reshape(KD, 128).T

    shared = {"wpack": np.ascontiguousarray(wpack),
              "vpack": np.ascontiguousarray(vpack)}
    in_maps = []
    for c in range(8):
        b, qh = c // 2, c % 2
        m = dict(shared)
        data = np.empty((128, 12 * D + 3 * KD), np.float32)
        xsl = x[b, qh * TQC:(qh + 1) * TQC, :]
        data[:, :NT * D] = xsl.reshape(NT, 128, D).transpose(1, 0, 2) \
                              .reshape(128, NT * D)
        data[:, NT * D:12 * D] = y[b].reshape(KD, 128, D) \
                                     .transpose(1, 0, 2).reshape(128, KD * D)
        data[:, 12 * D:12 * D + KD] = mask[b].astype(np.float32) \
                                             .reshape(KD, 128).T
        data[:, 12 * D + KD:12 * D + 2 * KD] = qb_col
        data[:, 12 * D + 2 * KD:] = kb_col
        m["data"] = data
        in_maps.append(m)
    return in_maps


def assemble(results):
    outf = np.empty((B, TQ, D), np.float32)
    for c in range(8):
        b, qh = c // 2, c % 2
        o = results[c]["out"]          # [128, NT, D] partition-major
        outf[b, qh * TQC:(qh + 1) * TQC, :] = \
            o.transpose(1, 0, 2).reshape(TQC, D)
    return outf


def kernel(**inputs) -> np.ndarray:
    nc = _get_nc()
    in_maps = make_in_maps(inputs)
    res = run_bass_kernel_spmd(nc, in_maps, list(range(8)))
    return assemble(res.results)


if __name__ == "__main__":
    nc = _get_nc()
    print("kernel built and compiled OK")


# revision 4
# speedup vs baseline: 10.1450x; 1.0274x over previous
"""CrossTransformer Trainium2 kernel, v2.

Shapes (hardcoded): B=4, TQ=TK=1024, D=1024, H=16, DK=DV=64.
Sharding: 8 cores = 4 batches x 2 query-row halves. Each core computes
out[b, qs:qs+512, :] independently (k/v work duplicated across the pair
of cores sharing a batch; no collectives).

v2 changes vs v1:
 - mask folded into V multiplicatively (masked tk rows of V and the
   ones-denominator column zeroed once) so the attention exp needs no
   per-tile bias and can run as large [128, 4x512] ACT ops straight
   from bf16 PSUM logits.
 - transposes batched 4-per-PSUM-bank with a single strided copy out.
 - LN affine runs on GpSimd in bf16; ReLU on ScalarE.
 - weight DMAs triple-buffered so they prefetch under compute.
 - optional For_i hardware loop (timing NEFFs run the body R times per
   dispatch to amortize host dispatch overhead).
"""
import sys

for _p in ("/root/.axon_site", "/root/.axon_site/_ro/trn_rl_repo",
           "/root/.axon_site/_ro/pypackages", "/opt/trn_rl_repo"):
    if _p not in sys.path:
        sys.path.append(_p)

import os
import numpy as np
import ml_dtypes
import concourse.bass as bass
from concourse import bacc
import concourse.tile as tile
import concourse.mybir as mybir
from concourse.masks import make_identity
from concourse.bass_utils import run_bass_kernel_spmd

F32 = mybir.dt.float32
BF = mybir.dt.bfloat16
AF = mybir.ActivationFunctionType
OP = mybir.AluOpType

RECIP_FAST = int(os.environ.get("K2_RECIP_FAST", "0"))
CAST_DMA = int(os.environ.get("K2_CAST_DMA", "1"))
AFFINE_DVE = int(os.environ.get("K2_AFFINE_DVE", "0"))

B, TQ, TK, D = 4, 1024, 1024, 1024
H, DK, DV = 16, 64, 64
TQC = TQ // 2          # 512 query rows per core
NT = TQC // 128        # 4 q-row tiles
KD = D // 128          # 8 contraction chunks

WEIGHT_NAMES = ["q_w", "k_w", "v_w", "o_w", "l1_w", "l2_w"]
BCAST_NAMES = ["v_b", "o_b", "l1_b", "l2_b",
               "ln1_g", "ln1_b", "ln2_g", "ln2_b",
               "mln1_g", "mln1_b", "mln2_g", "mln2_b"]


def build_kernel(compile=True, loop=0, repeat=1, stop_after=None):
    nc = bacc.Bacc()
    # host pre-arranges everything partition-major and packs the 23 inputs
    # into 3 tensors (each extra NEFF arg costs host dispatch time):
    #   data  [128, 12*D + 3*KD] f32: x tiles, y tiles, mask col, qb col, kb col
    #   wpack [128, 6, KD, D]   bf16: q,k,v,o,l1,l2 weights
    #   vpack [12, D]            f32: bias/gain vectors for broadcast
    data = nc.dram_tensor("data", (128, 12 * D + 3 * KD), F32,
                          kind="ExternalInput")
    wpack = nc.dram_tensor("wpack", (128, 6, KD, D), BF, kind="ExternalInput")
    vpack = nc.dram_tensor("vpack", (len(BCAST_NAMES), D), F32,
                           kind="ExternalInput")
    out = nc.dram_tensor("out", (128, NT, D), F32, kind="ExternalOutput")

    with tile.TileContext(nc) as tc:
        if loop:
            with tc.For_i(0, loop):
                _emit(nc, tc, data, wpack, vpack, out)
        else:
            for r in range(repeat):
                _emit(nc, tc, data, wpack, vpack, out,
                      pfx=f"r{r}_" if repeat > 1 else "",
                      stop_after=stop_after)
    if compile:
        nc.compile()
    return nc


def _emit(nc, tc, data, wpack, vpack, out, pfx="", stop_after=None):
    from contextlib import ExitStack

    ctx = ExitStack()
    with ctx:
        persist = ctx.enter_context(tc.tile_pool(name=pfx + "persist", bufs=1))
        lnp = ctx.enter_context(tc.tile_pool(name=pfx + "lnp", bufs=2))
        bcast = ctx.enter_context(tc.tile_pool(name=pfx + "bcast", bufs=4))
        wts = ctx.enter_context(tc.tile_pool(name=pfx + "wts", bufs=2))

        # ---------------- constants ----------------
        ident = persist.tile([128, 128], BF, tag="ident", name=pfx + "ident")
        make_identity(nc, ident)
        eps_t = persist.tile([128, 1], F32, tag="eps", name=pfx + "eps")
        nc.vector.memset(eps_t[:], 1e-5)
        ones_c = persist.tile([128, DV], BF, tag="ones_c", name=pfx + "ones_c")
        nc.vector.memset(ones_c[:], 1.0)
        # mask bits (1.0 keep / 0.0 drop) laid out [tk%128, tkblock]
        mbs = persist.tile([128, 3 * KD], F32, tag="mbs", name=pfx + "mbs")
        nc.sync.dma_start(mbs[:], data[:, 12 * D:])
        bq_sb = mbs[:, KD:2 * KD]
        bk_sb = mbs[:, 2 * KD:3 * KD]

        def bcast_tile(name, dt=F32):
            if not CAST_DMA:
                dt = F32
            i = BCAST_NAMES.index(name)
            t = bcast.tile([128, D], dt, tag="bc", name=pfx + f"bc_{name}")
            nc.gpsimd.dma_start(t[:],
                                vpack[i:i + 1, :].partition_broadcast(128))
            return t

        def load_weight(name):
            i = WEIGHT_NAMES.index(name)
            wt = wts.tile([128, KD, D], BF, tag="wbf", name=pfx + f"wbf_{name}")
            nc.sync.dma_start(wt[:], wpack[:, i, :, :])
            return wt

        # LN(+affine)+ReLU: src [128, D] f32 -> dst [128, D] bf16
        def ln_relu(src, gt, bt, dst, key, dve=False):
            stats = lnp.tile([128, 2, 6], F32, tag="stats", name=pfx + f"st_{key}")
            for i in range(2):
                nc.vector.bn_stats(stats[:, i, :], src[:, i * 512:(i + 1) * 512])
            mv = lnp.tile([128, 2], F32, tag="mv", name=pfx + f"mv_{key}")
            nc.vector.bn_aggr(mv[:], stats[:])
            std = lnp.tile([128, 1], F32, tag="std", name=pfx + f"sd_{key}")
            nc.scalar.activation(std[:], mv[:, 1:2], AF.Sqrt, bias=eps_t[:], scale=1.0)
            rstd = lnp.tile([128, 1], F32, tag="rstd", name=pfx + f"rs_{key}")
            nc.vector.reciprocal(rstd[:], std[:])
            z = lnp.tile([128, D], BF, tag="lnz", name=pfx + f"z_{key}")
            nc.vector.tensor_scalar(z[:], src[:], mv[:, 0:1], rstd[:],
                                    OP.subtract, OP.mult)
            if dve and AFFINE_DVE:
                nc.vector.tensor_tensor(z[:], z[:], gt[:], OP.mult)
                nc.vector.tensor_tensor(z[:], z[:], bt[:], OP.add)
                nc.vector.tensor_scalar_max(dst, z[:], 0.0)
            else:
                nc.gpsimd.tensor_tensor(z[:], z[:], gt[:], OP.mult)
                nc.gpsimd.tensor_tensor(z[:], z[:], bt[:], OP.add)
                nc.scalar.activation(dst, z[:], AF.Relu, scale=1.0)

        # transpose 4 [128,128] bf16 blocks through one PSUM bank, one copy out
        def transpose_quad(pstr, srcs, dst_ap, key):
            pt = pstr.tile([128, 4, 128], BF, tag="ps_tr", name=pfx + f"tr_{key}")
            for k, s in enumerate(srcs):
                nc.tensor.transpose(pt[:, k, :], s, ident[:])
            nc.vector.tensor_copy(dst_ap, pt[:])

        x_sb = persist.tile([128, NT, D], F32, tag="x_sb", name=pfx + "x_sb")
        qT = persist.tile([128, KD, TQC], BF, tag="qT", name=pfx + "qT")
        kT = persist.tile([128, KD, TK], BF, tag="kT", name=pfx + "kT")
        v_ext = persist.tile([128, KD, H, DV + 1], BF, tag="v_ext", name=pfx + "v_ext")
        attnT = persist.tile([128, KD, TQC], BF, tag="attnT", name=pfx + "attnT")

        ptr = ctx.enter_context(tc.tile_pool(name=pfx + "ptr", bufs=1))
        with (
            tc.tile_pool(name=pfx + "pab", bufs=3) as pab,
            tc.tile_pool(name=pfx + "pstr", bufs=2, space="PSUM") as pstr,
            tc.tile_pool(name=pfx + "psmC", bufs=4, space="PSUM") as psmm,
        ):
            # -------- phase A: LN + relu + transpose (x side, then y side)
            g1 = bcast_tile("ln1_g", BF)
            b1 = bcast_tile("ln1_b", BF)
            g2 = bcast_tile("ln2_g", BF)
            b2 = bcast_tile("ln2_b", BF)

            x1T = ptr.tile([128, KD, TQC], BF, tag="x1T", name=pfx + "x1T")
            for t in range(NT):
                nc.sync.dma_start(x_sb[:, t, :], data[:, t * D:(t + 1) * D])
                xz = pab.tile([128, D], BF, tag="xz", name=pfx + f"xz{t}")
                ln_relu(x_sb[:, t, :], g1, b1, xz[:], f"x{t}")
                for g in range(2):
                    transpose_quad(
                        pstr,
                        [xz[:, (g * 4 + k) * 128:(g * 4 + k + 1) * 128]
                         for k in range(4)],
                        x1T[:, g * 4:(g + 1) * 4, t * 128:(t + 1) * 128],
                        f"x{t}_{g}")

            y1T = ptr.tile([128, KD, TK], BF, tag="y1T", name=pfx + "y1T")
            for t in range(KD):
                yl = pab.tile([128, D], F32, tag="yload", name=pfx + f"yl_{t}")
                nc.scalar.dma_start(yl[:], data[:, (NT + t) * D:(NT + t + 1) * D])
                yz = pab.tile([128, D], BF, tag="yz", name=pfx + f"yz{t}")
                ln_relu(yl[:], g2, b2, yz[:], f"y{t}")
                for g in range(2):
                    transpose_quad(
                        pstr,
                        [yz[:, (g * 4 + k) * 128:(g * 4 + k + 1) * 128]
                         for k in range(4)],
                        y1T[:, g * 4:(g + 1) * 4, t * 128:(t + 1) * 128],
                        f"y{t}_{g}")

            if stop_after == "A":
                return
            # -------- phase C: projections --------
            # qT[hd, tq] = q_w.T @ x1T
            wq = load_weight("q_w")
            for m in range(KD):
                pq = psmm.tile([128, TQC], F32, tag="ps_mm", name=pfx + f"pq{m}")
                for kc in range(KD):
                    nc.tensor.matmul(pq[:], wq[:, kc, m * 128:(m + 1) * 128],
                                     x1T[:, kc, :],
                                     start=(kc == 0), stop=(kc == KD - 1))
                nc.scalar.activation(qT[:, m, :], pq[:], AF.Identity,
                                     bias=mbs[:, KD + m:KD + m + 1], scale=1.0)

            # kT[hd, tk] = k_w.T @ y1T
            wk = load_weight("k_w")
            for m in range(KD):
                for nt2 in range(2):
                    pk = psmm.tile([128, 512], F32, tag="ps_mm",
                                   name=pfx + f"pk{m}_{nt2}")
                    for kc in range(KD):
                        nc.tensor.matmul(pk[:], wk[:, kc, m * 128:(m + 1) * 128],
                                         y1T[:, kc, nt2 * 512:(nt2 + 1) * 512],
                                         start=(kc == 0), stop=(kc == KD - 1))
                    nc.scalar.activation(kT[:, m, nt2 * 512:(nt2 + 1) * 512], pk[:],
                                         AF.Identity, bias=mbs[:, 2 * KD + m:2 * KD + m + 1], scale=1.0)

            # v[tk, hdv] (+mask col) = y1 @ v_w ; then mask rows.
            # Only heads 0-7 (nt2=0) here; heads 8-15 are emitted inside
            # phase D after pair j=3, filling PE idle time while ACT drains
            # the exp backlog.
            wv = load_weight("v_w")
            bv = bcast_tile("v_b")
            nc.vector.memset(v_ext[:, :, :, DV:], 1.0)

            def emit_vproj(m, nt2, pv):
                for kc in range(KD):
                    nc.tensor.matmul(pv[:], y1T[:, kc, m * 128:(m + 1) * 128],
                                     wv[:, kc, nt2 * 512:(nt2 + 1) * 512],
                                     start=(kc == 0), stop=(kc == KD - 1))
                nc.vector.tensor_tensor(
                    v_ext[:, m, nt2 * 8:(nt2 + 1) * 8, :DV],
                    pv.rearrange("p (h v) -> p h v", v=DV),
                    bv[:, nt2 * 512:(nt2 + 1) * 512].rearrange(
                        "p (h v) -> p h v", v=DV),
                    OP.add)
                # zero masked tk rows (incl. the ones column -> denominator)
                nc.gpsimd.tensor_scalar_mul(
                    v_ext[:, m, nt2 * 8:(nt2 + 1) * 8, :]
                        .rearrange("p h v -> p (h v)"),
                    v_ext[:, m, nt2 * 8:(nt2 + 1) * 8, :]
                        .rearrange("p h v -> p (h v)"),
                    mbs[:, m:m + 1])

            for m in range(KD):
                pv = psmm.tile([128, 512], F32, tag="ps_mm", name=pfx + f"pv{m}_0")
                emit_vproj(m, 0, pv)

        if stop_after == "C":
            return
        # -------- phase D: attention (head pairs) --------
        # Rescale is batched: av results land in SBUF, the 8 denominator
        # rows of a 4-pair batch are DMA-gathered onto adjacent partitions
        # and divided with ONE reciprocal (HW reciprocal is ~8 cyc/elem,
        # so 16 separate [1,512] divides cost ~100us/iter).
        with (
            tc.tile_pool(name=pfx + "att", bufs=2) as att,
            tc.tile_pool(name=pfx + "psL", bufs=2, space="PSUM") as psL,
            tc.tile_pool(name=pfx + "psA", bufs=2, space="PSUM") as psA,
            tc.tile_pool(name=pfx + "psB", bufs=2, space="PSUM") as psB,
        ):
            # sel8[:, h, :] is a [8, DV] one-hot-row selector: row h is ones
            sel8 = persist.tile([8, 8, DV], BF, tag="sel8", name=pfx + "sel8")
            nc.vector.memset(sel8[:], 0.0)
            for k in range(8):
                nc.gpsimd.dma_start(sel8[k:k + 1, k, :], ones_c[0:1, :DV])
            for half in range(2):
                if half == 1:
                    for m in range(KD):
                        pv = psA.tile([128, TQC], F32, tag="ps_av",
                                      name=pfx + f"pv{m}_1")
                        emit_vproj(m, 1, pv)
                av_all = att.tile([128, 8, TQC], F32, tag="av_all",
                                  name=pfx + f"avh{half}", bufs=1)
                den8 = att.tile([8, TQC], F32, tag="den8", name=pfx + f"dn{half}")
                for jj in range(4):
                    j = half * 4 + jj
                    e_sb = att.tile([128, 2, KD, TQC], BF, tag="e_sb",
                                    name=pfx + f"e{j}")
                    for par in range(2):
                        hp = slice(par * 64, (par + 1) * 64)
                        for ch in range(4):
                            sl = psL.tile([128, 2, 512], F32, tag="ps_l",
                                          name=pfx + f"s{j}_{par}{ch}")
                            for k in range(2):
                                mt = ch * 2 + k
                                nc.tensor.matmul(sl[:, k, :],
                                                 kT[hp, j, mt * 128:(mt + 1) * 128],
                                                 qT[hp, j, :], start=True, stop=True)
                            nc.scalar.activation(
                                e_sb[:, par, ch * 2:(ch + 1) * 2, :], sl[:],
                                AF.Exp, scale=0.125)
                    if stop_after == "D1":
                        continue
                    for par in range(2):
                        h = 2 * j + par
                        hloc = 2 * jj + par
                        ps_av = psA.tile([128, TQC], F32, tag="ps_av",
                                         name=pfx + f"av{h}")
                        for kt in range(KD):
                            nc.tensor.matmul(ps_av[:DV + 1, :], v_ext[:, kt, h, :],
                                             e_sb[:, par, kt, :],
                                             start=(kt == 0), stop=(kt == KD - 1))
                        if stop_after == "D2":
                            nc.vector.tensor_copy(attnT[par * 64:par * 64 + DV, j, :],
                                                  ps_av[:DV, :])
                            continue
                        nc.vector.tensor_copy(av_all[:DV + 1, hloc, :],
                                              ps_av[:DV + 1, :])
                        nc.gpsimd.dma_start(den8[hloc:hloc + 1, :],
                                            av_all[DV:DV + 1, hloc, :])
                if stop_after in ("D1", "D2"):
                    continue
                rcp8 = att.tile([8, TQC], F32, tag="rcp8", name=pfx + f"rp{half}")
                nc.vector.reciprocal(rcp8[:], den8[:])
                rcb8 = att.tile([8, TQC], BF, tag="rcb8", name=pfx + f"rb{half}")
                nc.vector.tensor_copy(rcb8[:], rcp8[:])
                for jj in range(4):
                    j = half * 4 + jj
                    for par in range(2):
                        hloc = 2 * jj + par
                        oh = par * 64
                        ps_bc = psB.tile([DV, TQC], F32, tag="ps_bc",
                                         name=pfx + f"bc{half}_{hloc}")
                        nc.tensor.matmul(ps_bc[:], sel8[:, hloc, :], rcb8[:],
                                         start=True, stop=True)
                        nc.vector.tensor_tensor(attnT[oh:oh + DV, j, :],
                                                av_all[:DV, hloc, :],
                                                ps_bc[:], OP.mult)

        if stop_after in ("D", "D1", "D2"):
            return
        # -------- phase E: o-proj + residual --------
        psmE = ctx.enter_context(tc.tile_pool(name=pfx + "psmE", bufs=4,
                                              space="PSUM"))
        pstr2 = ctx.enter_context(tc.tile_pool(name=pfx + "pstr2", bufs=2,
                                               space="PSUM"))
        psmm = psmE
        wo = load_weight("o_w")
        bo = bcast_tile("o_b")
        for mt in range(NT):
            for nt2 in range(2):
                po = psmm.tile([128, 512], F32, tag="ps_mm", name=pfx + f"po{mt}_{nt2}")
                for kc in range(KD):
                    nc.tensor.matmul(po[:], attnT[:, kc, mt * 128:(mt + 1) * 128],
                                     wo[:, kc, nt2 * 512:(nt2 + 1) * 512],
                                     start=(kc == 0), stop=(kc == KD - 1))
                sl = slice(nt2 * 512, (nt2 + 1) * 512)
                nc.vector.tensor_tensor(x_sb[:, mt, sl], x_sb[:, mt, sl], po[:], OP.add)
                nc.gpsimd.tensor_tensor(x_sb[:, mt, sl], x_sb[:, mt, sl], bo[:, sl],
                                        OP.add)

        if stop_after == "E":
            return
        # -------- phases F/G: MLP --------
        with (
            tc.tile_pool(name=pfx + "mlp", bufs=1) as mlp,
            tc.tile_pool(name=pfx + "mz", bufs=2) as mz,
            tc.tile_pool(name=pfx + "mtr", bufs=1) as mtr,
        ):
            g3 = bcast_tile("mln1_g", BF)
            b3 = bcast_tile("mln1_b", BF)
            z1T = mtr.tile([128, KD, TQC], BF, tag="z1T", name=pfx + "z1T")
            for t in range(NT):
                z1 = mz.tile([128, D], BF, tag="z1", name=pfx + f"z1_{t}")
                ln_relu(x_sb[:, t, :], g3, b3, z1[:], f"z1_{t}", dve=True)
                for g in range(2):
                    transpose_quad(
                        pstr2,
                        [z1[:, (g * 4 + k) * 128:(g * 4 + k + 1) * 128]
                         for k in range(4)],
                        z1T[:, g * 4:(g + 1) * 4, t * 128:(t + 1) * 128],
                        f"z1{t}_{g}")
            w1 = load_weight("l1_w")
            bl1 = bcast_tile("l1_b")
            h_sb = mlp.tile([128, NT, D], F32, tag="h_sb", name=pfx + "h_sb")
            for mt in range(NT):
                for nt2 in range(2):
                    ph = psmm.tile([128, 512], F32, tag="ps_mm",
                                   name=pfx + f"ph{mt}_{nt2}")
                    for kc in range(KD):
                        nc.tensor.matmul(ph[:], z1T[:, kc, mt * 128:(mt + 1) * 128],
                                         w1[:, kc, nt2 * 512:(nt2 + 1) * 512],
                                         start=(kc == 0), stop=(kc == KD - 1))
                    sl = slice(nt2 * 512, (nt2 + 1) * 512)
                    nc.vector.tensor_tensor(h_sb[:, mt, sl], ph[:], bl1[:, sl], OP.add)

            if stop_after == "F":
                return
            g4 = bcast_tile("mln2_g", BF)
            b4 = bcast_tile("mln2_b", BF)
            z2T = mtr.tile([128, KD, TQC], BF, tag="z2T", name=pfx + "z2T")
            for t in range(NT):
                z2 = mz.tile([128, D], BF, tag="z2", name=pfx + f"z2_{t}")
                ln_relu(h_sb[:, t, :], g4, b4, z2[:], f"z2_{t}", dve=True)
                for g in range(2):
                    transpose_quad(
                        pstr2,
                        [z2[:, (g * 4 + k) * 128:(g * 4 + k + 1) * 128]
                         for k in range(4)],
                        z2T[:, g * 4:(g + 1) * 4, t * 128:(t + 1) * 128],
                        f"z2{t}_{g}")
            w2 = load_weight("l2_w")
            bl2 = bcast_tile("l2_b")
            out_r = out
            for mt in range(NT):
                o_sb = mlp.tile([128, D], F32, tag="o_sb", name=pfx + f"os{mt}")
                for nt2 in range(2):
                    pf = psmm.tile([128, 512], F32, tag="ps_mm",
                                   name=pfx + f"pf{mt}_{nt2}")
                    for kc in range(KD):
                        nc.tensor.matmul(pf[:], z2T[:, kc, mt * 128:(mt + 1) * 128],
                                         w2[:, kc, nt2 * 512:(nt2 + 1) * 512],
                                         start=(kc == 0), stop=(kc == KD - 1))
                    sl = slice(nt2 * 512, (nt2 + 1) * 512)
                    nc.vector.tensor_tensor(o_sb[:, sl], pf[:], bl2[:, sl], OP.add)
                nc.scalar.dma_start(out_r[:, mt, :], o_sb[:])


_NC_CACHE = None


def _get_nc():
    global _NC_CACHE
    if _NC_CACHE is None:
        _NC_CACHE = build_kernel()
    return _NC_CACHE


def make_in_maps(inputs):
    """Split full inputs into 8 per-core input maps (packed, partition-major)."""
    x = np.asarray(inputs["x"], np.float32)
    y = np.asarray(inputs["y"], np.float32)
    mask = np.asarray(inputs["mask"])

    wpack = np.empty((128, 6, KD, D), ml_dtypes.bfloat16)
    for i, n in enumerate(WEIGHT_NAMES):
        w = np.asarray(inputs[n], np.float32).astype(ml_dtypes.bfloat16)
        wpack[:, i] = w.reshape(KD, 128, D).transpose(1, 0, 2)
    vpack = np.stack([np.asarray(inputs[n], np.float32) for n in BCAST_NAMES])
    qb_col = np.asarray(inputs["q_b"], np.float32).reshape(KD, 128).T
    kb_col = np.asarray(inputs["k_b"], np.float32).reshape(KD, 128).T

    shared = {"wpack": np.ascontiguousarray(wpack),
              "vpack": np.ascontiguousarray(vpack)}
    in_maps = []
    for c in range(8):
        b, qh = c // 2, c % 2
        m = dict(shared)
        data = np.empty((128, 12 * D + 3 * KD), np.float32)
        xsl = x[b, qh * TQC:(qh + 1) * TQC, :]
        data[:, :NT * D] = xsl.reshape(NT, 128, D).transpose(1, 0, 2) \
                              .reshape(128, NT * D)
        data[:, NT * D:12 * D] = y[b].reshape(KD, 128, D) \
                                     .transpose(1, 0, 2).reshape(128, KD * D)
        data[:, 12 * D:12 * D + KD] = mask[b].astype(np.float32) \
                                             .reshape(KD, 128).T
        data[:, 12 * D + KD:12 * D + 2 * KD] = qb_col
        data[:, 12 * D + 2 * KD:] = kb_col
        m["data"] = data
        in_maps.append(m)
    return in_maps


def assemble(results):
    outf = np.empty((B, TQ, D), np.float32)
    for c in range(8):
        b, qh = c // 2, c % 2
        o = results[c]["out"]          # [128, NT, D] partition-major
        outf[b, qh * TQC:(qh + 1) * TQC, :] = \
            o.transpose(1, 0, 2).reshape(TQC, D)
    return outf


def kernel(**inputs) -> np.ndarray:
    nc = _get_nc()
    in_maps = make_in_maps(inputs)
    res = run_bass_kernel_spmd(nc, in_maps, list(range(8)))
    return assemble(res.results)


if __name__ == "__main__":
    nc = _get_nc()
    print("kernel built and compiled OK")


# revision 5
# speedup vs baseline: 10.4709x; 1.0321x over previous
"""CrossTransformer Trainium2 kernel, v2.

Shapes (hardcoded): B=4, TQ=TK=1024, D=1024, H=16, DK=DV=64.
Sharding: 8 cores = 4 batches x 2 query-row halves. Each core computes
out[b, qs:qs+512, :] independently (k/v work duplicated across the pair
of cores sharing a batch; no collectives).

v2 changes vs v1:
 - mask folded into V multiplicatively (masked tk rows of V and the
   ones-denominator column zeroed once) so the attention exp needs no
   per-tile bias and can run as large [128, 4x512] ACT ops straight
   from bf16 PSUM logits.
 - transposes batched 4-per-PSUM-bank with a single strided copy out.
 - LN affine runs on GpSimd in bf16; ReLU on ScalarE.
 - weight DMAs triple-buffered so they prefetch under compute.
 - optional For_i hardware loop (timing NEFFs run the body R times per
   dispatch to amortize host dispatch overhead).
"""
import sys

for _p in ("/root/.axon_site", "/root/.axon_site/_ro/trn_rl_repo",
           "/root/.axon_site/_ro/pypackages", "/opt/trn_rl_repo"):
    if _p not in sys.path:
        sys.path.append(_p)

import os
import numpy as np
import ml_dtypes
import concourse.bass as bass
from concourse import bacc
import concourse.tile as tile
import concourse.mybir as mybir
from concourse.masks import make_identity
from concourse.bass_utils import run_bass_kernel_spmd

F32 = mybir.dt.float32
BF = mybir.dt.bfloat16
AF = mybir.ActivationFunctionType
OP = mybir.AluOpType

RECIP_FAST = int(os.environ.get("K2_RECIP_FAST", "0"))
CAST_DMA = int(os.environ.get("K2_CAST_DMA", "1"))
AFFINE_DVE = int(os.environ.get("K2_AFFINE_DVE", "1"))

B, TQ, TK, D = 4, 1024, 1024, 1024
H, DK, DV = 16, 64, 64
TQC = TQ // 2          # 512 query rows per core
NT = TQC // 128        # 4 q-row tiles
KD = D // 128          # 8 contraction chunks

WEIGHT_NAMES = ["q_w", "k_w", "v_w", "o_w", "l1_w", "l2_w"]
BCAST_NAMES = ["v_b", "o_b", "l1_b", "l2_b",
               "ln1_g", "ln1_b", "ln2_g", "ln2_b",
               "mln1_g", "mln1_b", "mln2_g", "mln2_b"]


def build_kernel(compile=True, loop=0, repeat=1, stop_after=None):
    nc = bacc.Bacc()
    # host pre-arranges everything partition-major and packs the 23 inputs
    # into 3 tensors (each extra NEFF arg costs host dispatch time):
    #   data  [128, 12*D + 3*KD] f32: x tiles, y tiles, mask col, qb col, kb col
    #   wpack [128, 6, KD, D]   bf16: q,k,v,o,l1,l2 weights
    #   vpack [12, D]            f32: bias/gain vectors for broadcast
    data = nc.dram_tensor("data", (128, 12 * D + 3 * KD), F32,
                          kind="ExternalInput")
    wpack = nc.dram_tensor("wpack", (128, 6, KD, D), BF, kind="ExternalInput")
    vpack = nc.dram_tensor("vpack", (len(BCAST_NAMES), D), F32,
                           kind="ExternalInput")
    out = nc.dram_tensor("out", (128, NT, D), F32, kind="ExternalOutput")

    with tile.TileContext(nc) as tc:
        if loop:
            with tc.For_i(0, loop):
                _emit(nc, tc, data, wpack, vpack, out)
        else:
            for r in range(repeat):
                _emit(nc, tc, data, wpack, vpack, out,
                      pfx=f"r{r}_" if repeat > 1 else "",
                      stop_after=stop_after)
    if compile:
        nc.compile()
    return nc


def _emit(nc, tc, data, wpack, vpack, out, pfx="", stop_after=None):
    from contextlib import ExitStack

    ctx = ExitStack()
    with ctx:
        persist = ctx.enter_context(tc.tile_pool(name=pfx + "persist", bufs=1))
        lnp = ctx.enter_context(tc.tile_pool(name=pfx + "lnp", bufs=2))
        bcast = ctx.enter_context(tc.tile_pool(name=pfx + "bcast", bufs=4))
        wts = ctx.enter_context(tc.tile_pool(name=pfx + "wts", bufs=2))

        # ---------------- constants ----------------
        ident = persist.tile([128, 128], BF, tag="ident", name=pfx + "ident")
        make_identity(nc, ident)
        eps_t = persist.tile([128, 1], F32, tag="eps", name=pfx + "eps")
        nc.vector.memset(eps_t[:], 1e-5)
        ones_c = persist.tile([128, DV], BF, tag="ones_c", name=pfx + "ones_c")
        nc.vector.memset(ones_c[:], 1.0)
        # mask bits (1.0 keep / 0.0 drop) laid out [tk%128, tkblock]
        mbs = persist.tile([128, 3 * KD], F32, tag="mbs", name=pfx + "mbs")
        nc.sync.dma_start(mbs[:], data[:, 12 * D:])
        bq_sb = mbs[:, KD:2 * KD]
        bk_sb = mbs[:, 2 * KD:3 * KD]

        def bcast_tile(name, dt=F32):
            if not CAST_DMA:
                dt = F32
            i = BCAST_NAMES.index(name)
            t = bcast.tile([128, D], dt, tag="bc", name=pfx + f"bc_{name}")
            nc.gpsimd.dma_start(t[:],
                                vpack[i:i + 1, :].partition_broadcast(128))
            return t

        def load_weight(name):
            i = WEIGHT_NAMES.index(name)
            wt = wts.tile([128, KD, D], BF, tag="wbf", name=pfx + f"wbf_{name}")
            nc.sync.dma_start(wt[:], wpack[:, i, :, :])
            return wt

        # LN(+affine)+ReLU: src [128, D] f32 -> dst [128, D] bf16
        def ln_relu(src, gt, bt, dst, key, dve=False):
            stats = lnp.tile([128, 2, 6], F32, tag="stats", name=pfx + f"st_{key}")
            for i in range(2):
                nc.vector.bn_stats(stats[:, i, :], src[:, i * 512:(i + 1) * 512])
            mv = lnp.tile([128, 2], F32, tag="mv", name=pfx + f"mv_{key}")
            nc.vector.bn_aggr(mv[:], stats[:])
            std = lnp.tile([128, 1], F32, tag="std", name=pfx + f"sd_{key}")
            nc.scalar.activation(std[:], mv[:, 1:2], AF.Sqrt, bias=eps_t[:], scale=1.0)
            rstd = lnp.tile([128, 1], F32, tag="rstd", name=pfx + f"rs_{key}")
            nc.vector.reciprocal(rstd[:], std[:])
            z = lnp.tile([128, D], BF, tag="lnz", name=pfx + f"z_{key}")
            nc.vector.tensor_scalar(z[:], src[:], mv[:, 0:1], rstd[:],
                                    OP.subtract, OP.mult)
            if dve and AFFINE_DVE:
                nc.vector.tensor_tensor(z[:], z[:], gt[:], OP.mult)
                nc.vector.tensor_tensor(z[:], z[:], bt[:], OP.add)
                nc.vector.tensor_scalar_max(dst, z[:], 0.0)
            else:
                nc.gpsimd.tensor_tensor(z[:], z[:], gt[:], OP.mult)
                nc.gpsimd.tensor_tensor(z[:], z[:], bt[:], OP.add)
                nc.scalar.activation(dst, z[:], AF.Relu, scale=1.0)

        # transpose 4 [128,128] bf16 blocks through one PSUM bank, one copy out
        def transpose_quad(pstr, srcs, dst_ap, key):
            pt = pstr.tile([128, 4, 128], BF, tag="ps_tr", name=pfx + f"tr_{key}")
            for k, s in enumerate(srcs):
                nc.tensor.transpose(pt[:, k, :], s, ident[:])
            nc.vector.tensor_copy(dst_ap, pt[:])

        x_sb = persist.tile([128, NT, D], F32, tag="x_sb", name=pfx + "x_sb")
        qT = persist.tile([128, KD, TQC], BF, tag="qT", name=pfx + "qT")
        kT = persist.tile([128, KD, TK], BF, tag="kT", name=pfx + "kT")
        v_ext = persist.tile([128, KD, H, DV + 1], BF, tag="v_ext", name=pfx + "v_ext")
        attnT = persist.tile([128, KD, TQC], BF, tag="attnT", name=pfx + "attnT")

        ptr = ctx.enter_context(tc.tile_pool(name=pfx + "ptr", bufs=1))
        with (
            tc.tile_pool(name=pfx + "pab", bufs=3) as pab,
            tc.tile_pool(name=pfx + "pstr", bufs=2, space="PSUM") as pstr,
            tc.tile_pool(name=pfx + "psmC", bufs=4, space="PSUM") as psmm,
        ):
            # -------- phase A: LN + relu + transpose (x side, then y side)
            g1 = bcast_tile("ln1_g", BF)
            b1 = bcast_tile("ln1_b", BF)
            g2 = bcast_tile("ln2_g", BF)
            b2 = bcast_tile("ln2_b", BF)

            x1T = ptr.tile([128, KD, TQC], BF, tag="x1T", name=pfx + "x1T")
            for t in range(NT):
                nc.sync.dma_start(x_sb[:, t, :], data[:, t * D:(t + 1) * D])
                xz = pab.tile([128, D], BF, tag="xz", name=pfx + f"xz{t}")
                ln_relu(x_sb[:, t, :], g1, b1, xz[:], f"x{t}")
                for g in range(2):
                    transpose_quad(
                        pstr,
                        [xz[:, (g * 4 + k) * 128:(g * 4 + k + 1) * 128]
                         for k in range(4)],
                        x1T[:, g * 4:(g + 1) * 4, t * 128:(t + 1) * 128],
                        f"x{t}_{g}")

            y1T = ptr.tile([128, KD, TK], BF, tag="y1T", name=pfx + "y1T")
            for t in range(KD):
                yl = pab.tile([128, D], F32, tag="yload", name=pfx + f"yl_{t}")
                nc.scalar.dma_start(yl[:], data[:, (NT + t) * D:(NT + t + 1) * D])
                yz = pab.tile([128, D], BF, tag="yz", name=pfx + f"yz{t}")
                ln_relu(yl[:], g2, b2, yz[:], f"y{t}")
                for g in range(2):
                    transpose_quad(
                        pstr,
                        [yz[:, (g * 4 + k) * 128:(g * 4 + k + 1) * 128]
                         for k in range(4)],
                        y1T[:, g * 4:(g + 1) * 4, t * 128:(t + 1) * 128],
                        f"y{t}_{g}")

            if stop_after == "A":
                return
            # -------- phase C: projections --------
            # qT[hd, tq] = q_w.T @ x1T
            wq = load_weight("q_w")
            for m in range(KD):
                pq = psmm.tile([128, TQC], F32, tag="ps_mm", name=pfx + f"pq{m}")
                for kc in range(KD):
                    nc.tensor.matmul(pq[:], wq[:, kc, m * 128:(m + 1) * 128],
                                     x1T[:, kc, :],
                                     start=(kc == 0), stop=(kc == KD - 1))
                nc.vector.tensor_scalar_add(qT[:, m, :], pq[:],
                                            mbs[:, KD + m:KD + m + 1])

            # kT[hd, tk] = k_w.T @ y1T
            wk = load_weight("k_w")
            for m in range(KD):
                for nt2 in range(2):
                    pk = psmm.tile([128, 512], F32, tag="ps_mm",
                                   name=pfx + f"pk{m}_{nt2}")
                    for kc in range(KD):
                        nc.tensor.matmul(pk[:], wk[:, kc, m * 128:(m + 1) * 128],
                                         y1T[:, kc, nt2 * 512:(nt2 + 1) * 512],
                                         start=(kc == 0), stop=(kc == KD - 1))
                    nc.vector.tensor_scalar_add(
                        kT[:, m, nt2 * 512:(nt2 + 1) * 512], pk[:],
                        mbs[:, 2 * KD + m:2 * KD + m + 1])

            # v[tk, hdv] (+mask col) = y1 @ v_w ; then mask rows.
            # Only heads 0-7 (nt2=0) here; heads 8-15 are emitted inside
            # phase D after pair j=3, filling PE idle time while ACT drains
            # the exp backlog.
            wv = load_weight("v_w")
            bv = bcast_tile("v_b")
            nc.vector.memset(v_ext[:, :, :, DV:], 1.0)

            def emit_vproj(m, nt2, pv):
                for kc in range(KD):
                    nc.tensor.matmul(pv[:], y1T[:, kc, m * 128:(m + 1) * 128],
                                     wv[:, kc, nt2 * 512:(nt2 + 1) * 512],
                                     start=(kc == 0), stop=(kc == KD - 1))
                nc.vector.tensor_tensor(
                    v_ext[:, m, nt2 * 8:(nt2 + 1) * 8, :DV],
                    pv.rearrange("p (h v) -> p h v", v=DV),
                    bv[:, nt2 * 512:(nt2 + 1) * 512].rearrange(
                        "p (h v) -> p h v", v=DV),
                    OP.add)
                # zero masked tk rows (incl. the ones column -> denominator)
                nc.gpsimd.tensor_scalar_mul(
                    v_ext[:, m, nt2 * 8:(nt2 + 1) * 8, :]
                        .rearrange("p h v -> p (h v)"),
                    v_ext[:, m, nt2 * 8:(nt2 + 1) * 8, :]
                        .rearrange("p h v -> p (h v)"),
                    mbs[:, m:m + 1])

            for m in range(KD):
                pv = psmm.tile([128, 512], F32, tag="ps_mm", name=pfx + f"pv{m}_0")
                emit_vproj(m, 0, pv)

        if stop_after == "C":
            return
        # -------- phase D: attention (head pairs) --------
        # Rescale is batched: av results land in SBUF, the 8 denominator
        # rows of a 4-pair batch are DMA-gathered onto adjacent partitions
        # and divided with ONE reciprocal (HW reciprocal is ~8 cyc/elem,
        # so 16 separate [1,512] divides cost ~100us/iter).
        with (
            tc.tile_pool(name=pfx + "att", bufs=2) as att,
            tc.tile_pool(name=pfx + "psL", bufs=2, space="PSUM") as psL,
            tc.tile_pool(name=pfx + "psA", bufs=2, space="PSUM") as psA,
            tc.tile_pool(name=pfx + "psB", bufs=2, space="PSUM") as psB,
        ):
            # sel8[:, h, :] is a [8, DV] one-hot-row selector: row h is ones
            sel8 = persist.tile([8, 8, DV], BF, tag="sel8", name=pfx + "sel8")
            nc.vector.memset(sel8[:], 0.0)
            for k in range(8):
                nc.gpsimd.dma_start(sel8[k:k + 1, k, :], ones_c[0:1, :DV])
            for half in range(2):
                if half == 1:
                    for m in range(KD):
                        pv = psA.tile([128, TQC], F32, tag="ps_av",
                                      name=pfx + f"pv{m}_1")
                        emit_vproj(m, 1, pv)
                av_all = att.tile([128, 8, TQC], F32, tag="av_all",
                                  name=pfx + f"avh{half}", bufs=1)
                den8 = att.tile([8, TQC], F32, tag="den8", name=pfx + f"dn{half}")
                for jj in range(4):
                    j = half * 4 + jj
                    e_sb = att.tile([128, 2, KD, TQC], BF, tag="e_sb",
                                    name=pfx + f"e{j}")
                    for par in range(2):
                        hp = slice(par * 64, (par + 1) * 64)
                        for ch in range(4):
                            sl = psL.tile([128, 2, 512], F32, tag="ps_l",
                                          name=pfx + f"s{j}_{par}{ch}")
                            for k in range(2):
                                mt = ch * 2 + k
                                nc.tensor.matmul(sl[:, k, :],
                                                 kT[hp, j, mt * 128:(mt + 1) * 128],
                                                 qT[hp, j, :], start=True, stop=True)
                            nc.scalar.activation(
                                e_sb[:, par, ch * 2:(ch + 1) * 2, :], sl[:],
                                AF.Exp, scale=0.125)
                    if stop_after == "D1":
                        continue
                    for par in range(2):
                        h = 2 * j + par
                        hloc = 2 * jj + par
                        ps_av = psA.tile([128, TQC], F32, tag="ps_av",
                                         name=pfx + f"av{h}")
                        for kt in range(KD):
                            nc.tensor.matmul(ps_av[:DV + 1, :], v_ext[:, kt, h, :],
                                             e_sb[:, par, kt, :],
                                             start=(kt == 0), stop=(kt == KD - 1))
                        if stop_after == "D2":
                            nc.vector.tensor_copy(attnT[par * 64:par * 64 + DV, j, :],
                                                  ps_av[:DV, :])
                            continue
                        nc.vector.tensor_copy(av_all[:DV + 1, hloc, :],
                                              ps_av[:DV + 1, :])
                        nc.gpsimd.dma_start(den8[hloc:hloc + 1, :],
                                            av_all[DV:DV + 1, hloc, :])
                if stop_after in ("D1", "D2"):
                    continue
                rcp8 = att.tile([8, TQC], F32, tag="rcp8", name=pfx + f"rp{half}")
                nc.vector.reciprocal(rcp8[:], den8[:])
                rcb8 = att.tile([8, TQC], BF, tag="rcb8", name=pfx + f"rb{half}")
                nc.vector.tensor_copy(rcb8[:], rcp8[:])
                for jj in range(4):
                    j = half * 4 + jj
                    for par in range(2):
                        hloc = 2 * jj + par
                        oh = par * 64
                        ps_bc = psB.tile([DV, TQC], F32, tag="ps_bc",
                                         name=pfx + f"bc{half}_{hloc}")
                        nc.tensor.matmul(ps_bc[:], sel8[:, hloc, :], rcb8[:],
                                         start=True, stop=True)
                        nc.vector.tensor_tensor(attnT[oh:oh + DV, j, :],
                                                av_all[:DV, hloc, :],
                                                ps_bc[:], OP.mult)

        if stop_after in ("D", "D1", "D2"):
            return
        # -------- phase E: o-proj + residual --------
        psmE = ctx.enter_context(tc.tile_pool(name=pfx + "psmE", bufs=4,
                                              space="PSUM"))
        pstr2 = ctx.enter_context(tc.tile_pool(name=pfx + "pstr2", bufs=2,
                                               space="PSUM"))
        psmm = psmE
        wo = load_weight("o_w")
        bo = bcast_tile("o_b")
        for mt in range(NT):
            for nt2 in range(2):
                po = psmm.tile([128, 512], F32, tag="ps_mm", name=pfx + f"po{mt}_{nt2}")
                for kc in range(KD):
                    nc.tensor.matmul(po[:], attnT[:, kc, mt * 128:(mt + 1) * 128],
                                     wo[:, kc, nt2 * 512:(nt2 + 1) * 512],
                                     start=(kc == 0), stop=(kc == KD - 1))
                sl = slice(nt2 * 512, (nt2 + 1) * 512)
                nc.vector.tensor_tensor(x_sb[:, mt, sl], x_sb[:, mt, sl], po[:], OP.add)
                nc.gpsimd.tensor_tensor(x_sb[:, mt, sl], x_sb[:, mt, sl], bo[:, sl],
                                        OP.add)

        if stop_after == "E":
            return
        # -------- phases F/G: MLP --------
        with (
            tc.tile_pool(name=pfx + "mlp", bufs=1) as mlp,
            tc.tile_pool(name=pfx + "mz", bufs=2) as mz,
            tc.tile_pool(name=pfx + "mtr", bufs=1) as mtr,
        ):
            g3 = bcast_tile("mln1_g", BF)
            b3 = bcast_tile("mln1_b", BF)
            z1T = mtr.tile([128, KD, TQC], BF, tag="z1T", name=pfx + "z1T")
            for t in range(NT):
                z1 = mz.tile([128, D], BF, tag="z1", name=pfx + f"z1_{t}")
                ln_relu(x_sb[:, t, :], g3, b3, z1[:], f"z1_{t}", dve=True)
                for g in range(2):
                    transpose_quad(
                        pstr2,
                        [z1[:, (g * 4 + k) * 128:(g * 4 + k + 1) * 128]
                         for k in range(4)],
                        z1T[:, g * 4:(g + 1) * 4, t * 128:(t + 1) * 128],
                        f"z1{t}_{g}")
            w1 = load_weight("l1_w")
            bl1 = bcast_tile("l1_b")
            h_sb = mlp.tile([128, NT, D], F32, tag="h_sb", name=pfx + "h_sb")
            for mt in range(NT):
                for nt2 in range(2):
                    ph = psmm.tile([128, 512], F32, tag="ps_mm",
                                   name=pfx + f"ph{mt}_{nt2}")
                    for kc in range(KD):
                        nc.tensor.matmul(ph[:], z1T[:, kc, mt * 128:(mt + 1) * 128],
                                         w1[:, kc, nt2 * 512:(nt2 + 1) * 512],
                                         start=(kc == 0), stop=(kc == KD - 1))
                    sl = slice(nt2 * 512, (nt2 + 1) * 512)
                    nc.vector.tensor_tensor(h_sb[:, mt, sl], ph[:], bl1[:, sl], OP.add)

            if stop_after == "F":
                return
            g4 = bcast_tile("mln2_g", BF)
            b4 = bcast_tile("mln2_b", BF)
            z2T = mtr.tile([128, KD, TQC], BF, tag="z2T", name=pfx + "z2T")
            for t in range(NT):
                z2 = mz.tile([128, D], BF, tag="z2", name=pfx + f"z2_{t}")
                ln_relu(h_sb[:, t, :], g4, b4, z2[:], f"z2_{t}", dve=True)
                for g in range(2):
                    transpose_quad(
                        pstr2,
                        [z2[:, (g * 4 + k) * 128:(g * 4 + k + 1) * 128]
                         for k in range(4)],
                        z2T[:, g * 4:(g + 1) * 4, t * 128:(t + 1) * 128],
                        f"z2{t}_{g}")
            w2 = load_weight("l2_w")
            bl2 = bcast_tile("l2_b")
            out_r = out
            for mt in range(NT):
                o_sb = mlp.tile([128, D], F32, tag="o_sb", name=pfx + f"os{mt}")
                for nt2 in range(2):
                    pf = psmm.tile([128, 512], F32, tag="ps_mm",
                                   name=pfx + f"pf{mt}_{nt2}")
                    for kc in range(KD):
                        nc.tensor.matmul(pf[:], z2T[:, kc, mt * 128:(mt + 1) * 128],
                                         w2[:, kc, nt2 * 512:(nt2 + 1) * 512],
                                         start=(kc == 0), stop=(kc == KD - 1))
                    sl = slice(nt2 * 512, (nt2 + 1) * 512)
                    nc.vector.tensor_tensor(o_sb[:, sl], pf[:], bl2[:, sl], OP.add)
                nc.scalar.dma_start(out_r[:, mt, :], o_sb[:])


_NC_CACHE = None


def _get_nc():
    global _NC_CACHE
    if _NC_CACHE is None:
        _NC_CACHE = build_kernel()
    return _NC_CACHE


def make_in_maps(inputs):
    """Split full inputs into 8 per-core input maps (packed, partition-major)."""
    x = np.asarray(inputs["x"], np.float32)
    y = np.asarray(inputs["y"], np.float32)
    mask = np.asarray(inputs["mask"])

    wpack = np.empty((128, 6, KD, D), ml_dtypes.bfloat16)
    for i, n in enumerate(WEIGHT_NAMES):
        w = np.asarray(inputs[n], np.float32).astype(ml_dtypes.bfloat16)
        wpack[:, i] = w.reshape(KD, 128, D).transpose(1, 0, 2)
    vpack = np.stack([np.asarray(inputs[n], np.float32) for n in BCAST_NAMES])
    qb_col = np.asarray(inputs["q_b"], np.float32).reshape(KD, 128).T
    kb_col = np.asarray(inputs["k_b"], np.float32).reshape(KD, 128).T

    shared = {"wpack": np.ascontiguousarray(wpack),
              "vpack": np.ascontiguousarray(vpack)}
    in_maps = []
    for c in range(8):
        b, qh = c // 2, c % 2
        m = dict(shared)
        data = np.empty((128, 12 * D + 3 * KD), np.float32)
        xsl = x[b, qh * TQC:(qh + 1) * TQC, :]
        data[:, :NT * D] = xsl.reshape(NT, 128, D).transpose(1, 0, 2) \
                              .reshape(128, NT * D)
        data[:, NT * D:12 * D] = y[b].reshape(KD, 128, D) \
                                     .transpose(1, 0, 2).reshape(128, KD * D)
        data[:, 12 * D:12 * D + KD] = mask[b].astype(np.float32) \
                                             .reshape(KD, 128).T
        data[:, 12 * D + KD:12 * D + 2 * KD] = qb_col
        data[:, 12 * D + 2 * KD:] = kb_col
        m["data"] = data
        in_maps.append(m)
    return in_maps


def assemble(results):
    outf = np.empty((B, TQ, D), np.float32)
    for c in range(8):
        b, qh = c // 2, c % 2
        o = results[c]["out"]          # [128, NT, D] partition-major
        outf[b, qh * TQC:(qh + 1) * TQC, :] = \
            o.transpose(1, 0, 2).reshape(TQC, D)
    return outf


def kernel(**inputs) -> np.ndarray:
    nc = _get_nc()
    in_maps = make_in_maps(inputs)
    res = run_bass_kernel_spmd(nc, in_maps, list(range(8)))
    return assemble(res.results)


if __name__ == "__main__":
    nc = _get_nc()
    print("kernel built and compiled OK")
